# revision 23
# baseline (speedup 1.0000x reference)
"""Multi-head self-attention (B=2, T=2048, C=1024, H=16, causal, position bias)
on 8 Trainium2 NeuronCores.

Sharding: 2 heads per core (tensor parallel over heads), both batches on every
core. QKV projection computed per-core for its own head slice (x replicated,
pre-transposed on host). Attention fully per-core. Output projection is
token-sharded after an on-device AllToAll of the head-sharded attention
output; host concatenates the 8 token slices.

Numerics: all matmuls fp32r (~13-bit mantissa, full PE rate at N>=256).
The 1/scale = *8 is folded into Wq on the host; per-head causal bias max is
folded into the bias prep so exp() never overflows; causal mask is baked into
the pre-transposed bias tiles as -1e9.
"""
import numpy as np
import ml_dtypes

import concourse.bass as bass
import concourse.mybir as mybir
import concourse.tile as tile
from concourse import bacc
from concourse._compat import get_trn_type
from concourse.bass_utils import run_bass_kernel_spmd

F32 = mybir.dt.float32
F32R = mybir.dt.float32r
BF16 = mybir.dt.bfloat16
F16 = mybir.dt.float16
AF = mybir.ActivationFunctionType

N_CORES = 8
B = 2
T = 2048
C = 1024
H = 16
D = 64
HPC = H // N_CORES        # heads per core = 2
TQ = 128                  # query tile (layout A partitions)
KT = 128                  # key tile (layout B partitions)
QS = 512                  # query span (layout B free dim)
NSPAN = T // QS           # 4 spans per (b, head)
NEG = -1.0e9

_CACHE = {}


def _build():
    nc = bacc.Bacc(get_trn_type() or "TRN2", target_bir_lowering=False,
                   debug=False, num_devices=N_CORES)

    # ---- per-core DRAM parameters (contents differ per core) ----
    xT = nc.declare_dram_parameter("xT", [B, C, T], F32R, isOutput=False)           # x transposed
    wqkvT = nc.declare_dram_parameter("wqkvT", [C, 3 * 128], F32R, isOutput=False)  # [in, q8|k|v]
    biasT = nc.declare_dram_parameter("biasT", [HPC, T, T], F16, isOutput=False)   # masked, *8
    idf16 = nc.declare_dram_parameter("idf16", [128, 128], F16, isOutput=False)
    bmneg = nc.declare_dram_parameter("bmneg", [128, HPC], F32, isOutput=False)  # -8*bmax per head
    wprojT = nc.declare_dram_parameter("wprojT", [C, C], F32R, isOutput=False)      # W_proj.T
    id_r = nc.declare_dram_parameter("id_r", [128, 128], F32R, isOutput=False)      # identity
    id_f = nc.declare_dram_parameter("id_f", [128, 128], F32, isOutput=False)       # identity (transpose)
    id64x2 = nc.declare_dram_parameter("id64x2", [128, 64], F32, isOutput=False)  # [I64; I64]
    maskA16 = nc.declare_dram_parameter("maskA16", [128, 128], BF16, isOutput=False)  # strict-upper -1e9
    id16 = nc.declare_dram_parameter("id16", [128, 128], BF16, isOutput=False)
    ones_neg = nc.declare_dram_parameter("ones_neg", [1, 128], F32R, isOutput=False)  # all -1.0
    ones_col = nc.declare_dram_parameter("ones_col", [128, 16], F32R, isOutput=False)  # all 1.0
    ones_row = nc.declare_dram_parameter("ones_row", [1, T], F32R, isOutput=False)
    esel = nc.declare_dram_parameter("esel", [33, 128], F32R, isOutput=False)
    out = nc.declare_dram_parameter("out", [T * B // N_CORES, C], F32, isOutput=True)

    with tile.TileContext(nc) as tc:
        with (
            tc.tile_pool(name="consts", bufs=1) as consts,
            tc.tile_pool(name="wq", bufs=1) as wq_pool,
            tc.tile_pool(name="qkv", bufs=1) as qkv_pool,
            tc.tile_pool(name="stream", bufs=3) as stream,
            tc.tile_pool(name="bias", bufs=8) as bias_pool,
            tc.tile_pool(name="ptile", bufs=4) as p_pool,
            tc.tile_pool(name="yinp", bufs=1) as yinp,
            tc.tile_pool(name="stats", bufs=1) as stats,
            tc.tile_pool(name="ytile", bufs=1) as y_pool,
            tc.tile_pool(name="small", bufs=2) as small,
            tc.tile_pool(name="psA", bufs=3, space="PSUM") as psA,
            tc.tile_pool(name="psY", bufs=2, space="PSUM") as psY,
            tc.tile_pool(name="dram", bufs=1, space="DRAM") as dram,
        ):
            # ---------------- constants ----------------
            idr_t = consts.tile([128, 128], F32R, tag="idr")
            nc.sync.dma_start(idr_t[:], id_r[:])
            idf_t = consts.tile([128, 128], F32, tag="idf")
            nc.sync.dma_start(idf_t[:], id_f[:])
            maskA_t = consts.tile([128, 128], BF16, tag="maskA")
            nc.sync.dma_start(maskA_t[:], maskA16[:])
            id16_t = consts.tile([128, 128], BF16, tag="id16")
            nc.sync.dma_start(id16_t[:], id16[:])
            onesneg_t = consts.tile([1, 128], F32R, tag="onesneg")
            nc.sync.dma_start(onesneg_t[:], ones_neg[:])
            id64_t = consts.tile([128, 64], F32, tag="id64")
            nc.sync.dma_start(id64_t[:], id64x2[:])
            esel_t = consts.tile([33, 128], F32R, tag="esel")
            nc.sync.dma_start(esel_t[:], esel[:])
            idf16_t = consts.tile([128, 128], F16, tag="idf16")
            nc.sync.dma_start(idf16_t[:], idf16[:])
            bmneg_t = consts.tile([128, HPC], F32, tag="bmneg")
            nc.sync.dma_start(bmneg_t[:], bmneg[:])

            wqkv_t = wq_pool.tile([128, 8 * 384], F32R, tag="wqkv")
            for kk in range(8):
                nc.sync.dma_start(wqkv_t[:, kk * 384:(kk + 1) * 384],
                                  wqkvT[kk * 128:(kk + 1) * 128, :])

            # ---------------- phase 1: QKV projection ----------------
            # q8T/kT per (b, head): [65, 2048]; row 64: q8T = -mhat (per span),
            # kT = 1.0 (host). vT per b: [128 (2 heads), 2048].
            q8T = [[qkv_pool.tile([65, T], F32R, tag=f"q8T{b}{j}", name=f"q8T{b}{j}")
                    for j in range(HPC)] for b in range(B)]
            kTt = [[qkv_pool.tile([65, T], F32R, tag=f"kT{b}{j}", name=f"kT{b}{j}")
                    for j in range(HPC)] for b in range(B)]
            vTt = [qkv_pool.tile([128, T], F32R, tag=f"vT{b}", name=f"vT{b}")
                   for b in range(B)]
            for b in range(B):
                for j in range(HPC):
                    nc.sync.dma_start(kTt[b][j][64:65, :], ones_row[:, :])
            for b in range(B):
                for tp in range(2):
                    ps_m = [psA.tile([128, 1024], F32, tag="ps", name=f"psm{m_}")
                            for m_ in range(3)]
                    for kk in range(8):
                        xs = stream.tile([128, 1024], F32R, tag="xs")
                        nc.sync.dma_start(
                            xs[:], xT[b, kk * 128:(kk + 1) * 128,
                                      tp * 1024:(tp + 1) * 1024])
                        for m in range(3):
                            for u in range(2):
                                nc.tensor.matmul(
                                    ps_m[m][:, u * 512:(u + 1) * 512],
                                    wqkv_t[:, kk * 384 + m * 128: kk * 384 + (m + 1) * 128],
                                    xs[:, u * 512:(u + 1) * 512],
                                    start=(kk == 0), stop=(kk == 7))
                    cols = slice(tp * 1024, (tp + 1) * 1024)
                    for j in range(HPC):
                        nc.scalar.copy(q8T[b][j][0:64, cols],
                                       ps_m[0][64 * j:64 * (j + 1), :])
                        nc.scalar.copy(kTt[b][j][0:64, cols],
                                       ps_m[1][64 * j:64 * (j + 1), :])
                    nc.scalar.copy(vTt[b][:, cols], ps_m[2][:, :])

            # ---------------- phase 1b: v token-major + ones column ----------------
            v2 = [[y_pool.tile([128, 16 * 65], F32R, tag=f"v2_{b}{j}", name=f"v2_{b}{j}")
                   for j in range(HPC)] for b in range(B)]
            for b in range(B):
                for j in range(HPC):
                    nc.sync.dma_start(v2[b][j][:, 64::65], ones_col[:, :])
                    for kt in range(16):
                        pv = psA.tile([128, 1024], F32, tag="ps")
                        nc.tensor.transpose(
                            pv[:, 0:64],
                            vTt[b][64 * j:64 * (j + 1),
                                   kt * 128:(kt + 1) * 128].bitcast(F32),
                            id64_t[64 * j:64 * (j + 1), :])
                        nc.scalar.copy(v2[b][j][:, kt * 65:kt * 65 + 64], pv[:, 0:64])

            # ---------------- phase 2: attention ----------------
            a2a_in = [dram.tile([8, 128, 128], F32R, tag=f"a2a_in{q_}",
                                name=f"a2a_in{q_}") for q_ in range(NSPAN)]
            a2a_out = [dram.tile([8, 128, 128], F32R, tag=f"a2a_out{q_}",
                                 name=f"a2a_out{q_}") for q_ in range(NSPAN)]

            for Q in range(NSPAN):
                # ---- A-phase: -max(8 q.k) over valid keys -> q8T row 64 ----
                for b in range(B):
                    for j in range(HPC):
                        macc = stats.tile([128, 4], F32, tag=f"macc{b}{j}",
                                          name=f"macc{b}{j}")
                        for ii in range(4):
                            i = 4 * Q + ii
                            nkeys = (i + 1) * 128
                            nchunks = (nkeys + 511) // 512
                            for kc in range(nchunks):
                                n = min(512, nkeys - kc * 512)
                                pa = psA.tile([128, 1024], F32, tag="ps")
                                nc.tensor.matmul(
                                    pa[:, 0:n],
                                    q8T[b][j][0:64, i * 128:(i + 1) * 128],
                                    kTt[b][j][0:64, kc * 512:kc * 512 + n],
                                    start=True, stop=(kc != nchunks - 1))
                                if kc == nchunks - 1:
                                    dcol = nkeys - 128 - kc * 512
                                    nc.tensor.matmul(
                                        pa[:, dcol:dcol + 128],
                                        id16_t[:], maskA_t[:],
                                        start=False, stop=True)
                                if kc == 0:
                                    nc.vector.tensor_reduce(
                                        macc[:, ii:ii + 1], pa[:, 0:n],
                                        axis=mybir.AxisListType.X,
                                        op=mybir.AluOpType.max)
                                else:
                                    mtmp = small.tile([128, 1], F32, tag="mtmp")
                                    nc.vector.tensor_reduce(
                                        mtmp[:], pa[:, 0:n],
                                        axis=mybir.AxisListType.X,
                                        op=mybir.AluOpType.max)
                                    nc.vector.tensor_tensor(
                                        macc[:, ii:ii + 1],
                                        macc[:, ii:ii + 1], mtmp[:],
                                        op=mybir.AluOpType.max)
                        # negate, transpose [128,4]->[4,128], scatter into row 64
                        mneg = stats.tile([128, 4], F32, tag=f"mneg{b}{j}",
                                          name=f"mneg{b}{j}")
                        nc.vector.tensor_scalar(
                            mneg[:], macc[:], -1.0, bmneg_t[:, j:j + 1],
                            op0=mybir.AluOpType.mult, op1=mybir.AluOpType.add)
                        tp = psA.tile([128, 1024], F32, tag="ps")
                        nc.tensor.transpose(tp[0:4, 0:128], mneg[:], idf_t[:])
                        mtr = small.tile([4, 128], F32, tag="mtr")
                        nc.scalar.copy(mtr[:], tp[0:4, 0:128])
                        nc.gpsimd.dma_start(
                            q8T[b][j][64:65, Q * 512:(Q + 1) * 512]
                            .rearrange("o (t p) -> o t p", t=4),
                            mtr[:])

                if Q > 0:
                    nc.gpsimd.collective_compute(
                        "AllToAll", mybir.AluOpType.bypass,
                        replica_groups=[list(range(N_CORES))],
                        ins=[a2a_in[Q - 1].opt()], outs=[a2a_out[Q - 1].opt()])

                # ---- B-phase: scores^T (K=65 folds -mhat), exp, AV ----
                for j in range(HPC):
                    pY = {}
                    for b in range(B):
                        pY[b] = psY.tile([128, 512], F32, tag="psY",
                                         name=f"pY{b}{j}")
                    for kt2 in range(0, 4 * Q + 4, 2):
                        btp = bias_pool.tile([128, 1024], F16, tag="bias",
                                             name="btp")
                        for u_ in range(2):
                            nc.sync.dma_start(
                                btp[:, u_ * 512:(u_ + 1) * 512],
                                biasT[j, (kt2 + u_) * 128:(kt2 + u_ + 1) * 128,
                                      Q * 512:(Q + 1) * 512])
                        pb = {}
                        for b in range(B):
                            pb[b] = psA.tile([128, 1024], F32, tag="ps",
                                             name=f"pb{b}")
                        for u in range(2):
                            cols = slice(u * 512, (u + 1) * 512)
                            for b in range(B):
                                nc.tensor.matmul(
                                    pb[b][:, cols],
                                    kTt[b][j][:, (kt2 + u) * 128:(kt2 + u + 1) * 128],
                                    q8T[b][j][:, Q * 512:(Q + 1) * 512],
                                    start=True, stop=False)
                        for b in range(B):
                            for u in range(2):
                                cols = slice(u * 512, (u + 1) * 512)
                                nc.tensor.matmul(
                                    pb[b][:, cols], idf16_t[:], btp[:, cols],
                                    start=False, stop=True)
                        pt = {}
                        for b in range(B):
                            pt[b] = p_pool.tile([128, 1024], F32R, tag="p",
                                                name=f"pt{b}")
                            nc.scalar.activation(pt[b][:], pb[b][:], AF.Exp)
                        for b in range(B):
                            for u in range(2):
                                nc.tensor.matmul(
                                    pY[b][0:65, :],
                                    v2[b][j][:, (kt2 + u) * 65:(kt2 + u + 1) * 65],
                                    pt[b][:, u * 512:(u + 1) * 512],
                                    start=(kt2 + u == 0),
                                    stop=(kt2 + u == 4 * Q + 3))

                    # ---- normalize ----
                    for b in range(B):
                        linv = small.tile([1, 512], F32, tag="linv")
                        nc.vector.reciprocal(linv[:], pY[b][64:65, :])
                        linb = small.tile([64, 512], F32, tag="linb")
                        nc.gpsimd.partition_broadcast(linb[:], linv[:], channels=64)
                        ytmp = small.tile([64, 512], F32R, tag="ytmp")
                        nc.vector.tensor_tensor(
                            ytmp[:], pY[b][0:64, :], linb[:],
                            op=mybir.AluOpType.mult)
                        nc.sync.dma_start(
                            a2a_in[Q][:, 64 * j:64 * (j + 1),
                                      64 * b:64 * (b + 1)]
                            .rearrange("r c i -> c r i"),
                            ytmp[:].rearrange("c (r i) -> c r i", r=8))


            # ---------------- phase 4: output projection ----------------
            nc.gpsimd.collective_compute(
                "AllToAll", mybir.AluOpType.bypass,
                replica_groups=[list(range(N_CORES))],
                ins=[a2a_in[NSPAN - 1].opt()], outs=[a2a_out[NSPAN - 1].opt()])
            yin = [yinp.tile([128, 512], F32R, tag=f"yin{r}", name=f"yin{r}") for r in range(8)]
            for r in range(8):
                for q_ in range(NSPAN):
                    nc.sync.dma_start(yin[r][:, q_ * 128:(q_ + 1) * 128],
                                      a2a_out[q_][r])
            for oc in range(2):
                pp = [psA.tile([128, 1024], F32, tag="ps", name=f"pp{h_}")
                      for h_ in range(2)]
                for r in range(8):
                    wp = stream.tile([128, 512], F32R, tag="wp")
                    nc.sync.dma_start(
                        wp[:], wprojT[r * 128:(r + 1) * 128,
                                      oc * 512:(oc + 1) * 512])
                    for tt in range(4):
                        nc.tensor.matmul(
                            pp[tt // 2][:, (tt % 2) * 512:(tt % 2 + 1) * 512],
                            yin[r][:, tt * 128:(tt + 1) * 128],
                            wp[:],
                            start=(r == 0), stop=(r == 7))
                for tt in range(4):
                    ob = stream.tile([128, 512], F32, tag="ob")
                    nc.scalar.copy(ob[:], pp[tt // 2][:, (tt % 2) * 512:(tt % 2 + 1) * 512])
                    nc.sync.dma_start(
                        out[tt * 128:(tt + 1) * 128, oc * 512:(oc + 1) * 512], ob[:])

    nc.finalize()
    return nc


def _prep_inputs(x, position_bias, W_attn, W_proj):
    """Host-side shard/layout prep. Returns in_maps for the 8 cores."""
    x = np.asarray(x, np.float32)
    pb = np.asarray(position_bias, np.float32)[0]          # [H, T, T]
    W_attn = np.asarray(W_attn, np.float32)
    W_proj = np.asarray(W_proj, np.float32)

    xT = np.ascontiguousarray(x.transpose(0, 2, 1))        # [B, C, T]
    wprojT = np.ascontiguousarray(W_proj.T)                # [in, out]
    id_f = np.eye(128, dtype=np.float32)
    maskA = np.triu(np.full((128, 128), NEG, np.float32), 1)  # key>query -> -1e9
    id16 = np.eye(128, dtype=np.float32).astype(ml_dtypes.bfloat16)
    maskA16 = maskA.astype(ml_dtypes.bfloat16)
    ones_neg = np.full((1, 128), -1.0, np.float32)
    ones_col = np.ones((128, 16), np.float32)
    id64x2_np = np.vstack([np.eye(64, dtype=np.float32)] * 2)
    ones_row_np = np.ones((1, T), np.float32)
    esel_np = np.zeros((33, 128), np.float32)
    esel_np[0, 0:64] = 1.0
    esel_np[32, 64:128] = 1.0

    tril = np.tril(np.ones((T, T), dtype=bool))
    in_maps = []
    for c in range(N_CORES):
        wq = W_attn[128 * c:128 * (c + 1), :] * 8.0
        wk = W_attn[C + 128 * c:C + 128 * (c + 1), :]
        wv = W_attn[2 * C + 128 * c:2 * C + 128 * (c + 1), :]
        wqkvT = np.ascontiguousarray(np.concatenate([wq, wk, wv], 0).T)
        bt = np.empty((HPC, T, T), np.float16)
        bm = np.empty((HPC,), np.float32)
        for j in range(HPC):
            h = HPC * c + j
            bh = pb[h]
            bm[j] = -8.0 * float(bh[tril].max())
            btj = (8.0 * bh.T).astype(np.float16)          # [key, query]
            btj[~tril.T] = np.float16(-60000.0)            # key > query
            bt[j] = btj
        in_maps.append({
            "xT": xT, "wqkvT": wqkvT, "biasT": np.ascontiguousarray(bt),
            "wprojT": wprojT, "id_r": id_f, "id_f": id_f,
            "idf16": id_f.astype(np.float16),
            "bmneg": np.broadcast_to(bm, (128, HPC)).copy(),
            "maskA16": maskA16, "id16": id16, "id64x2": id64x2_np,
            "ones_neg": ones_neg, "ones_col": ones_col,
            "ones_row": ones_row_np, "esel": esel_np,
        })
    return in_maps


def kernel(x, position_bias, W_attn, W_proj, _trace=False, _tmpdir=None):
    if "nc" not in _CACHE:
        _CACHE["nc"] = _build()
    nc = _CACHE["nc"]
    in_maps = _prep_inputs(x, position_bias, W_attn, W_proj)
    res = run_bass_kernel_spmd(nc, in_maps, list(range(N_CORES)),
                               trace=_trace, tmpdir=_tmpdir)
    if _trace:
        _CACHE["exec_time_ns"] = res.exec_time_ns
    out_full = np.empty((B, T, C), np.float32)
    for c in range(N_CORES):
        r = res.results[c]["out"].reshape(NSPAN, B, 64, C)
        for b in range(B):
            for Qs in range(NSPAN):
                out_full[b, Qs * 512 + 64 * c: Qs * 512 + 64 * (c + 1)] = r[Qs, b]
    return out_full


# revision 25
# speedup vs baseline: 1.1222x; 1.1222x over previous
"""Multi-head self-attention (B=2, T=2048, C=1024, H=16, causal, position bias)
on 8 Trainium2 NeuronCores.

Sharding: 2 heads per core (tensor parallel over heads), both batches on every
core. QKV projection computed per-core for its own head slice (x replicated,
pre-transposed on host). Attention fully per-core. Output projection is
token-sharded after an on-device AllToAll of the head-sharded attention
output; host concatenates the 8 token slices.

Numerics: all matmuls fp32r (~13-bit mantissa, full PE rate at N>=256).
The 1/scale = *8 is folded into Wq on the host; per-head causal bias max is
folded into the bias prep so exp() never overflows; causal mask is baked into
the pre-transposed bias tiles as -1e9.
"""
import numpy as np
import ml_dtypes

import concourse.bass as bass
import concourse.mybir as mybir
import concourse.tile as tile
from concourse import bacc
from concourse._compat import get_trn_type
from concourse.bass_utils import run_bass_kernel_spmd

F32 = mybir.dt.float32
F32R = mybir.dt.float32r
BF16 = mybir.dt.bfloat16
F16 = mybir.dt.float16
AF = mybir.ActivationFunctionType

N_CORES = 8
B = 2
T = 2048
C = 1024
H = 16
D = 64
HPC = H // N_CORES        # heads per core = 2
TQ = 128                  # query tile (layout A partitions)
KT = 128                  # key tile (layout B partitions)
QS = 512                  # query span (layout B free dim)
NSPAN = T // QS           # 4 spans per (b, head)
NEG = -1.0e9

_CACHE = {}


def _build():
    nc = bacc.Bacc(get_trn_type() or "TRN2", target_bir_lowering=False,
                   debug=False, num_devices=N_CORES)

    # ---- per-core DRAM parameters (contents differ per core) ----
    xT = nc.declare_dram_parameter("xT", [B, C, T], F32R, isOutput=False)           # x transposed
    wqkvT = nc.declare_dram_parameter("wqkvT", [C, 3 * 128], F32R, isOutput=False)  # [in, q8|k|v]
    biasT = nc.declare_dram_parameter("biasT", [HPC, T, T], F16, isOutput=False)   # masked, *8
    idf16 = nc.declare_dram_parameter("idf16", [128, 128], F16, isOutput=False)
    bmneg = nc.declare_dram_parameter("bmneg", [128, HPC], F32, isOutput=False)  # -8*bmax per head
    wprojT = nc.declare_dram_parameter("wprojT", [C, C], F32R, isOutput=False)      # W_proj.T
    id_r = nc.declare_dram_parameter("id_r", [128, 128], F32R, isOutput=False)      # identity
    id_f = nc.declare_dram_parameter("id_f", [128, 128], F32, isOutput=False)       # identity (transpose)
    id64x2 = nc.declare_dram_parameter("id64x2", [128, 64], F32, isOutput=False)  # [I64; I64]
    maskAf = nc.declare_dram_parameter("maskAf", [128, 128], F32, isOutput=False)  # strict-upper -1e9
    id16 = nc.declare_dram_parameter("id16", [128, 128], BF16, isOutput=False)
    ones_neg = nc.declare_dram_parameter("ones_neg", [1, 128], F32R, isOutput=False)  # all -1.0
    ones_col = nc.declare_dram_parameter("ones_col", [128, 16], F32R, isOutput=False)  # all 1.0
    ones_row = nc.declare_dram_parameter("ones_row", [1, T], F32R, isOutput=False)
    esel = nc.declare_dram_parameter("esel", [33, 128], F32R, isOutput=False)
    out = nc.declare_dram_parameter("out", [T * B // N_CORES, C], F32, isOutput=True)

    with tile.TileContext(nc) as tc:
        with (
            tc.tile_pool(name="consts", bufs=1) as consts,
            tc.tile_pool(name="wq", bufs=1) as wq_pool,
            tc.tile_pool(name="qkv", bufs=1) as qkv_pool,
            tc.tile_pool(name="stream", bufs=3) as stream,
            tc.tile_pool(name="bias", bufs=8) as bias_pool,
            tc.tile_pool(name="ptile", bufs=4) as p_pool,
            tc.tile_pool(name="yinp", bufs=1) as yinp,
            tc.tile_pool(name="stats", bufs=1) as stats,
            tc.tile_pool(name="ytile", bufs=1) as y_pool,
            tc.tile_pool(name="small", bufs=2) as small,
            tc.tile_pool(name="psA", bufs=3, space="PSUM") as psA,
            tc.tile_pool(name="psY", bufs=2, space="PSUM") as psY,
            tc.tile_pool(name="dram", bufs=1, space="DRAM") as dram,
        ):
            # ---------------- constants ----------------
            idr_t = consts.tile([128, 128], F32R, tag="idr")
            nc.sync.dma_start(idr_t[:], id_r[:])
            idf_t = consts.tile([128, 128], F32, tag="idf")
            nc.sync.dma_start(idf_t[:], id_f[:])
            maskAf_t = consts.tile([128, 128], F32, tag="maskAf")
            nc.sync.dma_start(maskAf_t[:], maskAf[:])
            id16_t = consts.tile([128, 128], BF16, tag="id16")
            nc.sync.dma_start(id16_t[:], id16[:])
            onesneg_t = consts.tile([1, 128], F32R, tag="onesneg")
            nc.sync.dma_start(onesneg_t[:], ones_neg[:])
            id64_t = consts.tile([128, 64], F32, tag="id64")
            nc.sync.dma_start(id64_t[:], id64x2[:])
            esel_t = consts.tile([33, 128], F32R, tag="esel")
            nc.sync.dma_start(esel_t[:], esel[:])
            idf16_t = consts.tile([128, 128], F16, tag="idf16")
            nc.sync.dma_start(idf16_t[:], idf16[:])
            bmneg_t = consts.tile([128, HPC], F32, tag="bmneg")
            nc.sync.dma_start(bmneg_t[:], bmneg[:])

            wqkv_t = wq_pool.tile([128, 8 * 384], F32R, tag="wqkv")
            for kk in range(8):
                nc.sync.dma_start(wqkv_t[:, kk * 384:(kk + 1) * 384],
                                  wqkvT[kk * 128:(kk + 1) * 128, :])

            # ---------------- phase 1: QKV projection ----------------
            # q8T/kT per (b, head): [65, 2048]; row 64: q8T = -mhat (per span),
            # kT = 1.0 (host). vT per b: [128 (2 heads), 2048].
            q8T = [[qkv_pool.tile([65, T], F32R, tag=f"q8T{b}{j}", name=f"q8T{b}{j}")
                    for j in range(HPC)] for b in range(B)]
            kTt = [[qkv_pool.tile([65, T], F32R, tag=f"kT{b}{j}", name=f"kT{b}{j}")
                    for j in range(HPC)] for b in range(B)]
            vTt = [qkv_pool.tile([128, T], F32R, tag=f"vT{b}", name=f"vT{b}")
                   for b in range(B)]
            for b in range(B):
                for j in range(HPC):
                    nc.sync.dma_start(kTt[b][j][64:65, :], ones_row[:, :])
            for b in range(B):
                for tp in range(2):
                    ps_m = [psA.tile([128, 1024], F32, tag="ps", name=f"psm{m_}")
                            for m_ in range(3)]
                    for kk in range(8):
                        xs = stream.tile([128, 1024], F32R, tag="xs")
                        nc.sync.dma_start(
                            xs[:], xT[b, kk * 128:(kk + 1) * 128,
                                      tp * 1024:(tp + 1) * 1024])
                        for m in range(3):
                            for u in range(2):
                                nc.tensor.matmul(
                                    ps_m[m][:, u * 512:(u + 1) * 512],
                                    wqkv_t[:, kk * 384 + m * 128: kk * 384 + (m + 1) * 128],
                                    xs[:, u * 512:(u + 1) * 512],
                                    start=(kk == 0), stop=(kk == 7))
                    cols = slice(tp * 1024, (tp + 1) * 1024)
                    for j in range(HPC):
                        nc.scalar.copy(q8T[b][j][0:64, cols],
                                       ps_m[0][64 * j:64 * (j + 1), :])
                        nc.scalar.copy(kTt[b][j][0:64, cols],
                                       ps_m[1][64 * j:64 * (j + 1), :])
                    nc.scalar.copy(vTt[b][:, cols], ps_m[2][:, :])

            # ---------------- phase 1b: v token-major + ones column ----------------
            v2 = [[y_pool.tile([128, 16 * 65], F32R, tag=f"v2_{b}{j}", name=f"v2_{b}{j}")
                   for j in range(HPC)] for b in range(B)]
            for b in range(B):
                for j in range(HPC):
                    nc.sync.dma_start(v2[b][j][:, 64::65], ones_col[:, :])
                    for kt in range(16):
                        pv = psA.tile([128, 1024], F32, tag="ps")
                        nc.tensor.transpose(
                            pv[:, 0:64],
                            vTt[b][64 * j:64 * (j + 1),
                                   kt * 128:(kt + 1) * 128].bitcast(F32),
                            id64_t[64 * j:64 * (j + 1), :])
                        nc.scalar.copy(v2[b][j][:, kt * 65:kt * 65 + 64], pv[:, 0:64])

            # ---------------- phase 2: attention ----------------
            a2a_in = [dram.tile([8, 128, 128], F32R, tag=f"a2a_in{q_}",
                                name=f"a2a_in{q_}") for q_ in range(NSPAN)]
            a2a_out = [dram.tile([8, 128, 128], F32R, tag=f"a2a_out{q_}",
                                 name=f"a2a_out{q_}") for q_ in range(NSPAN)]

            for Q in range(NSPAN):
                # ---- A-phase: -max(8 q.k) over valid keys -> q8T row 64 ----
                for b in range(B):
                    for j in range(HPC):
                        macc = stats.tile([128, 4], F32, tag=f"macc{b}{j}",
                                          name=f"macc{b}{j}")
                        for ii in range(4):
                            i = 4 * Q + ii
                            nkeys = (i + 1) * 128
                            nchunks = (nkeys + 511) // 512
                            for kc in range(nchunks):
                                n = min(512, nkeys - kc * 512)
                                pa = psA.tile([128, 1024], F32, tag="ps")
                                nc.tensor.matmul(
                                    pa[:, 0:n],
                                    q8T[b][j][0:64, i * 128:(i + 1) * 128],
                                    kTt[b][j][0:64, kc * 512:kc * 512 + n],
                                    start=True, stop=True)
                                if kc == nchunks - 1:
                                    dcol = nkeys - 128 - kc * 512
                                    nc.vector.tensor_tensor(
                                        pa[:, dcol:dcol + 128],
                                        pa[:, dcol:dcol + 128], maskAf_t[:],
                                        op=mybir.AluOpType.add)
                                if kc == 0:
                                    nc.vector.tensor_reduce(
                                        macc[:, ii:ii + 1], pa[:, 0:n],
                                        axis=mybir.AxisListType.X,
                                        op=mybir.AluOpType.max)
                                else:
                                    mtmp = small.tile([128, 1], F32, tag="mtmp")
                                    nc.vector.tensor_reduce(
                                        mtmp[:], pa[:, 0:n],
                                        axis=mybir.AxisListType.X,
                                        op=mybir.AluOpType.max)
                                    nc.vector.tensor_tensor(
                                        macc[:, ii:ii + 1],
                                        macc[:, ii:ii + 1], mtmp[:],
                                        op=mybir.AluOpType.max)
                        # negate, transpose [128,4]->[4,128], scatter into row 64
                        mneg = stats.tile([128, 4], F32, tag=f"mneg{b}{j}",
                                          name=f"mneg{b}{j}")
                        nc.vector.tensor_scalar(
                            mneg[:], macc[:], -1.0, bmneg_t[:, j:j + 1],
                            op0=mybir.AluOpType.mult, op1=mybir.AluOpType.add)
                        tp = psA.tile([128, 1024], F32, tag="ps")
                        nc.tensor.transpose(tp[0:4, 0:128], mneg[:], idf_t[:])
                        mtr = small.tile([4, 128], F32, tag="mtr")
                        nc.scalar.copy(mtr[:], tp[0:4, 0:128])
                        nc.gpsimd.dma_start(
                            q8T[b][j][64:65, Q * 512:(Q + 1) * 512]
                            .rearrange("o (t p) -> o t p", t=4),
                            mtr[:])

                if Q > 0:
                    nc.gpsimd.collective_compute(
                        "AllToAll", mybir.AluOpType.bypass,
                        replica_groups=[list(range(N_CORES))],
                        ins=[a2a_in[Q - 1].opt()], outs=[a2a_out[Q - 1].opt()])

                # ---- B-phase: scores^T (K=65 folds -mhat), exp, AV ----
                for j in range(HPC):
                    pY = {}
                    for b in range(B):
                        pY[b] = psY.tile([128, 512], F32, tag="psY",
                                         name=f"pY{b}{j}")
                    for kt2 in range(0, 4 * Q + 4, 2):
                        btp = bias_pool.tile([128, 1024], F16, tag="bias",
                                             name="btp")
                        for u_ in range(2):
                            nc.sync.dma_start(
                                btp[:, u_ * 512:(u_ + 1) * 512],
                                biasT[j, (kt2 + u_) * 128:(kt2 + u_ + 1) * 128,
                                      Q * 512:(Q + 1) * 512])
                        for b in range(B):
                            pb = psA.tile([128, 1024], F32, tag="ps")
                            for u in range(2):
                                cols = slice(u * 512, (u + 1) * 512)
                                nc.tensor.matmul(
                                    pb[:, cols],
                                    kTt[b][j][:, (kt2 + u) * 128:(kt2 + u + 1) * 128],
                                    q8T[b][j][:, Q * 512:(Q + 1) * 512],
                                    start=True, stop=True)
                            nc.vector.tensor_tensor(
                                pb[:], pb[:], btp[:], op=mybir.AluOpType.add)
                            pt = p_pool.tile([128, 1024], F32R, tag="p")
                            nc.scalar.activation(pt[:], pb[:], AF.Exp)
                            for u in range(2):
                                nc.tensor.matmul(
                                    pY[b][0:65, :],
                                    v2[b][j][:, (kt2 + u) * 65:(kt2 + u + 1) * 65],
                                    pt[:, u * 512:(u + 1) * 512],
                                    start=(kt2 + u == 0),
                                    stop=(kt2 + u == 4 * Q + 3))

                    # ---- normalize ----
                    for b in range(B):
                        linv = small.tile([1, 512], F32, tag="linv")
                        nc.vector.reciprocal(linv[:], pY[b][64:65, :])
                        linb = small.tile([64, 512], F32, tag="linb")
                        nc.gpsimd.partition_broadcast(linb[:], linv[:], channels=64)
                        ytmp = small.tile([64, 512], F32R, tag="ytmp")
                        nc.vector.tensor_tensor(
                            ytmp[:], pY[b][0:64, :], linb[:],
                            op=mybir.AluOpType.mult)
                        nc.sync.dma_start(
                            a2a_in[Q][:, 64 * j:64 * (j + 1),
                                      64 * b:64 * (b + 1)]
                            .rearrange("r c i -> c r i"),
                            ytmp[:].rearrange("c (r i) -> c r i", r=8))


            # ---------------- phase 4: output projection ----------------
            nc.gpsimd.collective_compute(
                "AllToAll", mybir.AluOpType.bypass,
                replica_groups=[list(range(N_CORES))],
                ins=[a2a_in[NSPAN - 1].opt()], outs=[a2a_out[NSPAN - 1].opt()])
            yin = [yinp.tile([128, 512], F32R, tag=f"yin{r}", name=f"yin{r}") for r in range(8)]
            for r in range(8):
                for q_ in range(NSPAN):
                    nc.sync.dma_start(yin[r][:, q_ * 128:(q_ + 1) * 128],
                                      a2a_out[q_][r])
            for oc in range(2):
                pp = [psA.tile([128, 1024], F32, tag="ps", name=f"pp{h_}")
                      for h_ in range(2)]
                for r in range(8):
                    wp = stream.tile([128, 512], F32R, tag="wp")
                    nc.sync.dma_start(
                        wp[:], wprojT[r * 128:(r + 1) * 128,
                                      oc * 512:(oc + 1) * 512])
                    for tt in range(4):
                        nc.tensor.matmul(
                            pp[tt // 2][:, (tt % 2) * 512:(tt % 2 + 1) * 512],
                            yin[r][:, tt * 128:(tt + 1) * 128],
                            wp[:],
                            start=(r == 0), stop=(r == 7))
                for tt in range(4):
                    ob = stream.tile([128, 512], F32, tag="ob")
                    nc.scalar.copy(ob[:], pp[tt // 2][:, (tt % 2) * 512:(tt % 2 + 1) * 512])
                    nc.sync.dma_start(
                        out[tt * 128:(tt + 1) * 128, oc * 512:(oc + 1) * 512], ob[:])

    nc.finalize()
    return nc


def _prep_inputs(x, position_bias, W_attn, W_proj):
    """Host-side shard/layout prep. Returns in_maps for the 8 cores."""
    x = np.asarray(x, np.float32)
    pb = np.asarray(position_bias, np.float32)[0]          # [H, T, T]
    W_attn = np.asarray(W_attn, np.float32)
    W_proj = np.asarray(W_proj, np.float32)

    xT = np.ascontiguousarray(x.transpose(0, 2, 1))        # [B, C, T]
    wprojT = np.ascontiguousarray(W_proj.T)                # [in, out]
    id_f = np.eye(128, dtype=np.float32)
    maskA = np.triu(np.full((128, 128), NEG, np.float32), 1)  # key>query -> -1e9
    id16 = np.eye(128, dtype=np.float32).astype(ml_dtypes.bfloat16)
    maskA16 = maskA.astype(ml_dtypes.bfloat16)
    ones_neg = np.full((1, 128), -1.0, np.float32)
    ones_col = np.ones((128, 16), np.float32)
    id64x2_np = np.vstack([np.eye(64, dtype=np.float32)] * 2)
    ones_row_np = np.ones((1, T), np.float32)
    esel_np = np.zeros((33, 128), np.float32)
    esel_np[0, 0:64] = 1.0
    esel_np[32, 64:128] = 1.0

    tril = np.tril(np.ones((T, T), dtype=bool))
    in_maps = []
    for c in range(N_CORES):
        wq = W_attn[128 * c:128 * (c + 1), :] * 8.0
        wk = W_attn[C + 128 * c:C + 128 * (c + 1), :]
        wv = W_attn[2 * C + 128 * c:2 * C + 128 * (c + 1), :]
        wqkvT = np.ascontiguousarray(np.concatenate([wq, wk, wv], 0).T)
        bt = np.empty((HPC, T, T), np.float16)
        bm = np.empty((HPC,), np.float32)
        for j in range(HPC):
            h = HPC * c + j
            bh = pb[h]
            bm[j] = -8.0 * float(bh[tril].max())
            btj = (8.0 * bh.T).astype(np.float16)          # [key, query]
            btj[~tril.T] = np.float16(-60000.0)            # key > query
            bt[j] = btj
        in_maps.append({
            "xT": xT, "wqkvT": wqkvT, "biasT": np.ascontiguousarray(bt),
            "wprojT": wprojT, "id_r": id_f, "id_f": id_f,
            "idf16": id_f.astype(np.float16),
            "bmneg": np.broadcast_to(bm, (128, HPC)).copy(),
            "maskAf": maskA, "id16": id16, "id64x2": id64x2_np,
            "ones_neg": ones_neg, "ones_col": ones_col,
            "ones_row": ones_row_np, "esel": esel_np,
        })
    return in_maps


def kernel(x, position_bias, W_attn, W_proj, _trace=False, _tmpdir=None):
    if "nc" not in _CACHE:
        _CACHE["nc"] = _build()
    nc = _CACHE["nc"]
    in_maps = _prep_inputs(x, position_bias, W_attn, W_proj)
    res = run_bass_kernel_spmd(nc, in_maps, list(range(N_CORES)),
                               trace=_trace, tmpdir=_tmpdir)
    if _trace:
        _CACHE["exec_time_ns"] = res.exec_time_ns
    out_full = np.empty((B, T, C), np.float32)
    for c in range(N_CORES):
        r = res.results[c]["out"].reshape(NSPAN, B, 64, C)
        for b in range(B):
            for Qs in range(NSPAN):
                out_full[b, Qs * 512 + 64 * c: Qs * 512 + 64 * (c + 1)] = r[Qs, b]
    return out_full


# revision 26
# speedup vs baseline: 1.1493x; 1.0241x over previous
"""Multi-head self-attention (B=2, T=2048, C=1024, H=16, causal, position bias)
on 8 Trainium2 NeuronCores.

Sharding: 2 heads per core (tensor parallel over heads), both batches on every
core. QKV projection computed per-core for its own head slice (x replicated,
pre-transposed on host). Attention fully per-core. Output projection is
token-sharded after an on-device AllToAll of the head-sharded attention
output; host concatenates the 8 token slices.

Numerics: all matmuls fp32r (~13-bit mantissa, full PE rate at N>=256).
The 1/scale = *8 is folded into Wq on the host; per-head causal bias max is
folded into the bias prep so exp() never overflows; causal mask is baked into
the pre-transposed bias tiles as -1e9.
"""
import numpy as np
import ml_dtypes

import concourse.bass as bass
import concourse.mybir as mybir
import concourse.tile as tile
from concourse import bacc
from concourse._compat import get_trn_type
from concourse.bass_utils import run_bass_kernel_spmd

F32 = mybir.dt.float32
F32R = mybir.dt.float32r
BF16 = mybir.dt.bfloat16
F16 = mybir.dt.float16
AF = mybir.ActivationFunctionType

N_CORES = 8
B = 2
T = 2048
C = 1024
H = 16
D = 64
HPC = H // N_CORES        # heads per core = 2
TQ = 128                  # query tile (layout A partitions)
KT = 128                  # key tile (layout B partitions)
QS = 512                  # query span (layout B free dim)
NSPAN = T // QS           # 4 spans per (b, head)
NEG = -1.0e9

_CACHE = {}


def _build():
    nc = bacc.Bacc(get_trn_type() or "TRN2", target_bir_lowering=False,
                   debug=False, num_devices=N_CORES)

    # ---- per-core DRAM parameters (contents differ per core) ----
    xT = nc.declare_dram_parameter("xT", [B, C, T], F32R, isOutput=False)           # x transposed
    wqkvT = nc.declare_dram_parameter("wqkvT", [C, 3 * 128], F32R, isOutput=False)  # [in, q8|k|v]
    biasT = nc.declare_dram_parameter("biasT", [HPC, T, T], F16, isOutput=False)   # masked, *8
    idf16 = nc.declare_dram_parameter("idf16", [128, 128], F16, isOutput=False)
    bmneg = nc.declare_dram_parameter("bmneg", [128, HPC], F32, isOutput=False)  # -8*bmax per head
    wprojT = nc.declare_dram_parameter("wprojT", [C, C], F32R, isOutput=False)      # W_proj.T
    id_r = nc.declare_dram_parameter("id_r", [128, 128], F32R, isOutput=False)      # identity
    id_f = nc.declare_dram_parameter("id_f", [128, 128], F32, isOutput=False)       # identity (transpose)
    id64x2 = nc.declare_dram_parameter("id64x2", [128, 64], F32, isOutput=False)  # [I64; I64]
    maskA16 = nc.declare_dram_parameter("maskA16", [128, 128], BF16, isOutput=False)  # strict-upper -1e9
    id16 = nc.declare_dram_parameter("id16", [128, 128], BF16, isOutput=False)
    ones_neg = nc.declare_dram_parameter("ones_neg", [1, 128], F32R, isOutput=False)  # all -1.0
    ones_col = nc.declare_dram_parameter("ones_col", [128, 16], F32R, isOutput=False)  # all 1.0
    ones_row = nc.declare_dram_parameter("ones_row", [1, T], F32R, isOutput=False)
    esel = nc.declare_dram_parameter("esel", [33, 128], F32R, isOutput=False)
    out = nc.declare_dram_parameter("out", [T * B // N_CORES, C], F32, isOutput=True)

    with tile.TileContext(nc) as tc:
        with (
            tc.tile_pool(name="consts", bufs=1) as consts,
            tc.tile_pool(name="wq", bufs=1) as wq_pool,
            tc.tile_pool(name="qkv", bufs=1) as qkv_pool,
            tc.tile_pool(name="stream", bufs=3) as stream,
            tc.tile_pool(name="bias", bufs=8) as bias_pool,
            tc.tile_pool(name="ptile", bufs=4) as p_pool,
            tc.tile_pool(name="yinp", bufs=1) as yinp,
            tc.tile_pool(name="stats", bufs=1) as stats,
            tc.tile_pool(name="ytile", bufs=1) as y_pool,
            tc.tile_pool(name="small", bufs=2) as small,
            tc.tile_pool(name="psA", bufs=3, space="PSUM") as psA,
            tc.tile_pool(name="psY", bufs=2, space="PSUM") as psY,
            tc.tile_pool(name="dram", bufs=1, space="DRAM") as dram,
        ):
            # ---------------- constants ----------------
            idr_t = consts.tile([128, 128], F32R, tag="idr")
            nc.sync.dma_start(idr_t[:], id_r[:])
            idf_t = consts.tile([128, 128], F32, tag="idf")
            nc.sync.dma_start(idf_t[:], id_f[:])
            maskA_t = consts.tile([128, 128], BF16, tag="maskA")
            nc.sync.dma_start(maskA_t[:], maskA16[:])
            id16_t = consts.tile([128, 128], BF16, tag="id16")
            nc.sync.dma_start(id16_t[:], id16[:])
            onesneg_t = consts.tile([1, 128], F32R, tag="onesneg")
            nc.sync.dma_start(onesneg_t[:], ones_neg[:])
            id64_t = consts.tile([128, 64], F32, tag="id64")
            nc.sync.dma_start(id64_t[:], id64x2[:])
            esel_t = consts.tile([33, 128], F32R, tag="esel")
            nc.sync.dma_start(esel_t[:], esel[:])
            idf16_t = consts.tile([128, 128], F16, tag="idf16")
            nc.sync.dma_start(idf16_t[:], idf16[:])
            bmneg_t = consts.tile([128, HPC], F32, tag="bmneg")
            nc.sync.dma_start(bmneg_t[:], bmneg[:])

            wqkv_t = wq_pool.tile([128, 8 * 384], F32R, tag="wqkv")
            for kk in range(8):
                nc.sync.dma_start(wqkv_t[:, kk * 384:(kk + 1) * 384],
                                  wqkvT[kk * 128:(kk + 1) * 128, :])

            # ---------------- phase 1: QKV projection ----------------
            # q8T/kT per (b, head): [65, 2048]; row 64: q8T = -mhat (per span),
            # kT = 1.0 (host). vT per b: [128 (2 heads), 2048].
            q8T = [[qkv_pool.tile([65, T], F32R, tag=f"q8T{b}{j}", name=f"q8T{b}{j}")
                    for j in range(HPC)] for b in range(B)]
            kTt = [[qkv_pool.tile([65, T], F32R, tag=f"kT{b}{j}", name=f"kT{b}{j}")
                    for j in range(HPC)] for b in range(B)]
            vTt = [qkv_pool.tile([128, T], F32R, tag=f"vT{b}", name=f"vT{b}")
                   for b in range(B)]
            for b in range(B):
                for j in range(HPC):
                    nc.sync.dma_start(kTt[b][j][64:65, :], ones_row[:, :])
            for b in range(B):
                for tp in range(2):
                    ps_m = [psA.tile([128, 1024], F32, tag="ps", name=f"psm{m_}")
                            for m_ in range(3)]
                    for kk in range(8):
                        xs = stream.tile([128, 1024], F32R, tag="xs")
                        nc.sync.dma_start(
                            xs[:], xT[b, kk * 128:(kk + 1) * 128,
                                      tp * 1024:(tp + 1) * 1024])
                        for m in range(3):
                            for u in range(2):
                                nc.tensor.matmul(
                                    ps_m[m][:, u * 512:(u + 1) * 512],
                                    wqkv_t[:, kk * 384 + m * 128: kk * 384 + (m + 1) * 128],
                                    xs[:, u * 512:(u + 1) * 512],
                                    start=(kk == 0), stop=(kk == 7))
                    cols = slice(tp * 1024, (tp + 1) * 1024)
                    for j in range(HPC):
                        nc.scalar.copy(q8T[b][j][0:64, cols],
                                       ps_m[0][64 * j:64 * (j + 1), :])
                        nc.scalar.copy(kTt[b][j][0:64, cols],
                                       ps_m[1][64 * j:64 * (j + 1), :])
                    nc.scalar.copy(vTt[b][:, cols], ps_m[2][:, :])

            # ---------------- phase 1b: v token-major + ones column ----------------
            v2 = [[y_pool.tile([128, 16 * 65], F32R, tag=f"v2_{b}{j}", name=f"v2_{b}{j}")
                   for j in range(HPC)] for b in range(B)]
            for b in range(B):
                for j in range(HPC):
                    nc.sync.dma_start(v2[b][j][:, 64::65], ones_col[:, :])
                    for kt in range(16):
                        pv = psA.tile([128, 1024], F32, tag="ps")
                        nc.tensor.transpose(
                            pv[:, 0:64],
                            vTt[b][64 * j:64 * (j + 1),
                                   kt * 128:(kt + 1) * 128].bitcast(F32),
                            id64_t[64 * j:64 * (j + 1), :])
                        nc.scalar.copy(v2[b][j][:, kt * 65:kt * 65 + 64], pv[:, 0:64])

            # ---------------- phase 2: attention ----------------
            a2a_in = [dram.tile([8, 128, 128], F32R, tag=f"a2a_in{q_}",
                                name=f"a2a_in{q_}") for q_ in range(NSPAN)]
            a2a_out = [dram.tile([8, 128, 128], F32R, tag=f"a2a_out{q_}",
                                 name=f"a2a_out{q_}") for q_ in range(NSPAN)]

            for Q in range(NSPAN):
                # ---- A-phase: -max(8 q.k) over valid keys -> q8T row 64 ----
                for b in range(B):
                    for j in range(HPC):
                        macc = stats.tile([128, 4], F32, tag=f"macc{b}{j}",
                                          name=f"macc{b}{j}")
                        for ii in range(4):
                            i = 4 * Q + ii
                            nkeys = (i + 1) * 128
                            nchunks = (nkeys + 1023) // 1024
                            for kc in range(nchunks):
                                n = min(1024, nkeys - kc * 1024)
                                pa = psA.tile([128, 1024], F32, tag="ps")
                                for u_ in range(0, n, 512):
                                    nn = min(512, n - u_)
                                    nc.tensor.matmul(
                                        pa[:, u_:u_ + nn],
                                        q8T[b][j][0:64, i * 128:(i + 1) * 128],
                                        kTt[b][j][0:64,
                                                  kc * 1024 + u_:kc * 1024 + u_ + nn],
                                        start=True,
                                        stop=not (kc == nchunks - 1
                                                  and nkeys - 128 >= kc * 1024 + u_
                                                  and nkeys - 128 < kc * 1024 + u_ + nn))
                                if kc == nchunks - 1:
                                    dcol = nkeys - 128 - kc * 1024
                                    nc.tensor.matmul(
                                        pa[:, dcol:dcol + 128],
                                        id16_t[:], maskA_t[:],
                                        start=False, stop=True)
                                if kc == 0:
                                    nc.vector.tensor_reduce(
                                        macc[:, ii:ii + 1], pa[:, 0:n],
                                        axis=mybir.AxisListType.X,
                                        op=mybir.AluOpType.max)
                                else:
                                    mtmp = small.tile([128, 1], F32, tag="mtmp")
                                    nc.vector.tensor_reduce(
                                        mtmp[:], pa[:, 0:n],
                                        axis=mybir.AxisListType.X,
                                        op=mybir.AluOpType.max)
                                    nc.vector.tensor_tensor(
                                        macc[:, ii:ii + 1],
                                        macc[:, ii:ii + 1], mtmp[:],
                                        op=mybir.AluOpType.max)
                        # negate, transpose [128,4]->[4,128], scatter into row 64
                        mneg = stats.tile([128, 4], F32, tag=f"mneg{b}{j}",
                                          name=f"mneg{b}{j}")
                        nc.vector.tensor_scalar(
                            mneg[:], macc[:], -1.0, bmneg_t[:, j:j + 1],
                            op0=mybir.AluOpType.mult, op1=mybir.AluOpType.add)
                        tp = psA.tile([128, 1024], F32, tag="ps")
                        nc.tensor.transpose(tp[0:4, 0:128], mneg[:], idf_t[:])
                        mtr = small.tile([4, 128], F32, tag="mtr")
                        nc.scalar.copy(mtr[:], tp[0:4, 0:128])
                        nc.gpsimd.dma_start(
                            q8T[b][j][64:65, Q * 512:(Q + 1) * 512]
                            .rearrange("o (t p) -> o t p", t=4),
                            mtr[:])

                if Q > 0:
                    nc.gpsimd.collective_compute(
                        "AllToAll", mybir.AluOpType.bypass,
                        replica_groups=[list(range(N_CORES))],
                        ins=[a2a_in[Q - 1].opt()], outs=[a2a_out[Q - 1].opt()])

                # ---- B-phase: scores^T (K=65 folds -mhat), exp, AV ----
                for j in range(HPC):
                    pY = {}
                    for b in range(B):
                        pY[b] = psY.tile([128, 512], F32, tag="psY",
                                         name=f"pY{b}{j}")
                    for kt2 in range(0, 4 * Q + 4, 2):
                        btp = bias_pool.tile([128, 1024], F16, tag="bias",
                                             name="btp")
                        for u_ in range(2):
                            nc.sync.dma_start(
                                btp[:, u_ * 512:(u_ + 1) * 512],
                                biasT[j, (kt2 + u_) * 128:(kt2 + u_ + 1) * 128,
                                      Q * 512:(Q + 1) * 512])
                        for b in range(B):
                            pb = psA.tile([128, 1024], F32, tag="ps")
                            for u in range(2):
                                cols = slice(u * 512, (u + 1) * 512)
                                nc.tensor.matmul(
                                    pb[:, cols],
                                    kTt[b][j][:, (kt2 + u) * 128:(kt2 + u + 1) * 128],
                                    q8T[b][j][:, Q * 512:(Q + 1) * 512],
                                    start=True, stop=True)
                            nc.vector.tensor_tensor(
                                pb[:], pb[:], btp[:], op=mybir.AluOpType.add)
                            pt = p_pool.tile([128, 1024], F32R, tag="p")
                            nc.scalar.activation(pt[:], pb[:], AF.Exp)
                            for u in range(2):
                                nc.tensor.matmul(
                                    pY[b][0:65, :],
                                    v2[b][j][:, (kt2 + u) * 65:(kt2 + u + 1) * 65],
                                    pt[:, u * 512:(u + 1) * 512],
                                    start=(kt2 + u == 0),
                                    stop=(kt2 + u == 4 * Q + 3))

                    # ---- normalize ----
                    for b in range(B):
                        linv = small.tile([1, 512], F32, tag="linv")
                        nc.vector.reciprocal(linv[:], pY[b][64:65, :])
                        linb = small.tile([64, 512], F32, tag="linb")
                        nc.gpsimd.partition_broadcast(linb[:], linv[:], channels=64)
                        ytmp = small.tile([64, 512], F32R, tag="ytmp")
                        nc.vector.tensor_tensor(
                            ytmp[:], pY[b][0:64, :], linb[:],
                            op=mybir.AluOpType.mult)
                        nc.sync.dma_start(
                            a2a_in[Q][:, 64 * j:64 * (j + 1),
                                      64 * b:64 * (b + 1)]
                            .rearrange("r c i -> c r i"),
                            ytmp[:].rearrange("c (r i) -> c r i", r=8))


            # ---------------- phase 4: output projection ----------------
            nc.gpsimd.collective_compute(
                "AllToAll", mybir.AluOpType.bypass,
                replica_groups=[list(range(N_CORES))],
                ins=[a2a_in[NSPAN - 1].opt()], outs=[a2a_out[NSPAN - 1].opt()])
            yin = [yinp.tile([128, 512], F32R, tag=f"yin{r}", name=f"yin{r}") for r in range(8)]
            for r in range(8):
                for q_ in range(NSPAN):
                    nc.sync.dma_start(yin[r][:, q_ * 128:(q_ + 1) * 128],
                                      a2a_out[q_][r])
            for oc in range(2):
                pp = [psA.tile([128, 1024], F32, tag="ps", name=f"pp{h_}")
                      for h_ in range(2)]
                for r in range(8):
                    wp = stream.tile([128, 512], F32R, tag="wp")
                    nc.sync.dma_start(
                        wp[:], wprojT[r * 128:(r + 1) * 128,
                                      oc * 512:(oc + 1) * 512])
                    for tt in range(4):
                        nc.tensor.matmul(
                            pp[tt // 2][:, (tt % 2) * 512:(tt % 2 + 1) * 512],
                            yin[r][:, tt * 128:(tt + 1) * 128],
                            wp[:],
                            start=(r == 0), stop=(r == 7))
                for tt in range(4):
                    ob = stream.tile([128, 512], F32, tag="ob")
                    nc.scalar.copy(ob[:], pp[tt // 2][:, (tt % 2) * 512:(tt % 2 + 1) * 512])
                    nc.sync.dma_start(
                        out[tt * 128:(tt + 1) * 128, oc * 512:(oc + 1) * 512], ob[:])

    nc.finalize()
    return nc


def _prep_inputs(x, position_bias, W_attn, W_proj):
    """Host-side shard/layout prep. Returns in_maps for the 8 cores."""
    x = np.asarray(x, np.float32)
    pb = np.asarray(position_bias, np.float32)[0]          # [H, T, T]
    W_attn = np.asarray(W_attn, np.float32)
    W_proj = np.asarray(W_proj, np.float32)

    xT = np.ascontiguousarray(x.transpose(0, 2, 1))        # [B, C, T]
    wprojT = np.ascontiguousarray(W_proj.T)                # [in, out]
    id_f = np.eye(128, dtype=np.float32)
    maskA = np.triu(np.full((128, 128), NEG, np.float32), 1)  # key>query -> -1e9
    id16 = np.eye(128, dtype=np.float32).astype(ml_dtypes.bfloat16)
    maskA16_np = maskA.astype(ml_dtypes.bfloat16)
    ones_neg = np.full((1, 128), -1.0, np.float32)
    ones_col = np.ones((128, 16), np.float32)
    id64x2_np = np.vstack([np.eye(64, dtype=np.float32)] * 2)
    ones_row_np = np.ones((1, T), np.float32)
    esel_np = np.zeros((33, 128), np.float32)
    esel_np[0, 0:64] = 1.0
    esel_np[32, 64:128] = 1.0

    tril = np.tril(np.ones((T, T), dtype=bool))
    in_maps = []
    for c in range(N_CORES):
        wq = W_attn[128 * c:128 * (c + 1), :] * 8.0
        wk = W_attn[C + 128 * c:C + 128 * (c + 1), :]
        wv = W_attn[2 * C + 128 * c:2 * C + 128 * (c + 1), :]
        wqkvT = np.ascontiguousarray(np.concatenate([wq, wk, wv], 0).T)
        bt = np.empty((HPC, T, T), np.float16)
        bm = np.empty((HPC,), np.float32)
        for j in range(HPC):
            h = HPC * c + j
            bh = pb[h]
            bm[j] = -8.0 * float(bh[tril].max())
            btj = (8.0 * bh.T).astype(np.float16)          # [key, query]
            btj[~tril.T] = np.float16(-60000.0)            # key > query
            bt[j] = btj
        in_maps.append({
            "xT": xT, "wqkvT": wqkvT, "biasT": np.ascontiguousarray(bt),
            "wprojT": wprojT, "id_r": id_f, "id_f": id_f,
            "idf16": id_f.astype(np.float16),
            "bmneg": np.broadcast_to(bm, (128, HPC)).copy(),
            "maskA16": maskA16_np, "id16": id16, "id64x2": id64x2_np,
            "ones_neg": ones_neg, "ones_col": ones_col,
            "ones_row": ones_row_np, "esel": esel_np,
        })
    return in_maps


def kernel(x, position_bias, W_attn, W_proj, _trace=False, _tmpdir=None):
    if "nc" not in _CACHE:
        _CACHE["nc"] = _build()
    nc = _CACHE["nc"]
    in_maps = _prep_inputs(x, position_bias, W_attn, W_proj)
    res = run_bass_kernel_spmd(nc, in_maps, list(range(N_CORES)),
                               trace=_trace, tmpdir=_tmpdir)
    if _trace:
        _CACHE["exec_time_ns"] = res.exec_time_ns
    out_full = np.empty((B, T, C), np.float32)
    for c in range(N_CORES):
        r = res.results[c]["out"].reshape(NSPAN, B, 64, C)
        for b in range(B):
            for Qs in range(NSPAN):
                out_full[b, Qs * 512 + 64 * c: Qs * 512 + 64 * (c + 1)] = r[Qs, b]
    return out_full


# revision 27
# speedup vs baseline: 1.1621x; 1.0112x over previous
"""Multi-head self-attention (B=2, T=2048, C=1024, H=16, causal, position bias)
on 8 Trainium2 NeuronCores.

Sharding: 2 heads per core (tensor parallel over heads), both batches on every
core. QKV projection computed per-core for its own head slice (x replicated,
pre-transposed on host). Attention fully per-core. Output projection is
token-sharded after an on-device AllToAll of the head-sharded attention
output; host concatenates the 8 token slices.

Numerics: all matmuls fp32r (~13-bit mantissa, full PE rate at N>=256).
The 1/scale = *8 is folded into Wq on the host; per-head causal bias max is
folded into the bias prep so exp() never overflows; causal mask is baked into
the pre-transposed bias tiles as -1e9.
"""
import numpy as np
import ml_dtypes

import concourse.bass as bass
import concourse.mybir as mybir
import concourse.tile as tile
from concourse import bacc
from concourse._compat import get_trn_type
from concourse.bass_utils import run_bass_kernel_spmd

F32 = mybir.dt.float32
F32R = mybir.dt.float32r
BF16 = mybir.dt.bfloat16
F16 = mybir.dt.float16
AF = mybir.ActivationFunctionType

N_CORES = 8
B = 2
T = 2048
C = 1024
H = 16
D = 64
HPC = H // N_CORES        # heads per core = 2
TQ = 128                  # query tile (layout A partitions)
KT = 128                  # key tile (layout B partitions)
QS = 512                  # query span (layout B free dim)
NSPAN = T // QS           # 4 spans per (b, head)
NEG = -1.0e9

_CACHE = {}


def _build():
    nc = bacc.Bacc(get_trn_type() or "TRN2", target_bir_lowering=False,
                   debug=False, num_devices=N_CORES)

    # ---- per-core DRAM parameters (contents differ per core) ----
    xT = nc.declare_dram_parameter("xT", [B, C, T], F32R, isOutput=False)           # x transposed
    wqkvT = nc.declare_dram_parameter("wqkvT", [C, 3 * 128], F32R, isOutput=False)  # [in, q8|k|v]
    biasT = nc.declare_dram_parameter("biasT", [HPC, T, T], F16, isOutput=False)   # masked, *8
    idf16 = nc.declare_dram_parameter("idf16", [128, 128], F16, isOutput=False)
    bmneg = nc.declare_dram_parameter("bmneg", [128, HPC], F32, isOutput=False)  # -8*bmax per head
    wprojT = nc.declare_dram_parameter("wprojT", [C, C], F32R, isOutput=False)      # W_proj.T
    id_r = nc.declare_dram_parameter("id_r", [128, 128], F32R, isOutput=False)      # identity
    id_f = nc.declare_dram_parameter("id_f", [128, 128], F32, isOutput=False)       # identity (transpose)
    id64x2 = nc.declare_dram_parameter("id64x2", [128, 64], F32, isOutput=False)  # [I64; I64]
    maskA16 = nc.declare_dram_parameter("maskA16", [128, 128], BF16, isOutput=False)  # strict-upper -1e9
    id16 = nc.declare_dram_parameter("id16", [128, 128], BF16, isOutput=False)
    ones_neg = nc.declare_dram_parameter("ones_neg", [1, 128], F32R, isOutput=False)  # all -1.0
    ones_col = nc.declare_dram_parameter("ones_col", [128, 16], BF16, isOutput=False)  # all 1.0
    ones_row = nc.declare_dram_parameter("ones_row", [1, T], F32R, isOutput=False)
    esel = nc.declare_dram_parameter("esel", [33, 128], F32R, isOutput=False)
    out = nc.declare_dram_parameter("out", [T * B // N_CORES, C], F32, isOutput=True)

    with tile.TileContext(nc) as tc:
        with (
            tc.tile_pool(name="consts", bufs=1) as consts,
            tc.tile_pool(name="wq", bufs=1) as wq_pool,
            tc.tile_pool(name="qkv", bufs=1) as qkv_pool,
            tc.tile_pool(name="stream", bufs=3) as stream,
            tc.tile_pool(name="bias", bufs=8) as bias_pool,
            tc.tile_pool(name="ptile", bufs=4) as p_pool,
            tc.tile_pool(name="yinp", bufs=1) as yinp,
            tc.tile_pool(name="stats", bufs=1) as stats,
            tc.tile_pool(name="ytile", bufs=1) as y_pool,
            tc.tile_pool(name="small", bufs=2) as small,
            tc.tile_pool(name="psA", bufs=3, space="PSUM") as psA,
            tc.tile_pool(name="psY", bufs=2, space="PSUM") as psY,
            tc.tile_pool(name="dram", bufs=1, space="DRAM") as dram,
        ):
            # ---------------- constants ----------------
            idr_t = consts.tile([128, 128], F32R, tag="idr")
            nc.sync.dma_start(idr_t[:], id_r[:])
            idf_t = consts.tile([128, 128], F32, tag="idf")
            nc.sync.dma_start(idf_t[:], id_f[:])
            maskA_t = consts.tile([128, 128], BF16, tag="maskA")
            nc.sync.dma_start(maskA_t[:], maskA16[:])
            id16_t = consts.tile([128, 128], BF16, tag="id16")
            nc.sync.dma_start(id16_t[:], id16[:])
            onesneg_t = consts.tile([1, 128], F32R, tag="onesneg")
            nc.sync.dma_start(onesneg_t[:], ones_neg[:])
            id64_t = consts.tile([128, 64], F32, tag="id64")
            nc.sync.dma_start(id64_t[:], id64x2[:])
            esel_t = consts.tile([33, 128], F32R, tag="esel")
            nc.sync.dma_start(esel_t[:], esel[:])
            idf16_t = consts.tile([128, 128], F16, tag="idf16")
            nc.sync.dma_start(idf16_t[:], idf16[:])
            bmneg_t = consts.tile([128, HPC], F32, tag="bmneg")
            nc.sync.dma_start(bmneg_t[:], bmneg[:])

            wqkv_t = wq_pool.tile([128, 8 * 384], F32R, tag="wqkv")
            for kk in range(8):
                nc.sync.dma_start(wqkv_t[:, kk * 384:(kk + 1) * 384],
                                  wqkvT[kk * 128:(kk + 1) * 128, :])

            # ---------------- phase 1: QKV projection ----------------
            # q8T/kT per (b, head): [65, 2048]; row 64: q8T = -mhat (per span),
            # kT = 1.0 (host). vT per b: [128 (2 heads), 2048].
            q8T = [[qkv_pool.tile([65, T], F32R, tag=f"q8T{b}{j}", name=f"q8T{b}{j}")
                    for j in range(HPC)] for b in range(B)]
            kTt = [[qkv_pool.tile([65, T], F32R, tag=f"kT{b}{j}", name=f"kT{b}{j}")
                    for j in range(HPC)] for b in range(B)]
            vTt = [qkv_pool.tile([128, T], F32R, tag=f"vT{b}", name=f"vT{b}")
                   for b in range(B)]
            for b in range(B):
                for j in range(HPC):
                    nc.sync.dma_start(kTt[b][j][64:65, :], ones_row[:, :])
            for b in range(B):
                for tp in range(2):
                    ps_m = [psA.tile([128, 1024], F32, tag="ps", name=f"psm{m_}")
                            for m_ in range(3)]
                    for kk in range(8):
                        xs = stream.tile([128, 1024], F32R, tag="xs")
                        nc.sync.dma_start(
                            xs[:], xT[b, kk * 128:(kk + 1) * 128,
                                      tp * 1024:(tp + 1) * 1024])
                        for m in range(3):
                            for u in range(2):
                                nc.tensor.matmul(
                                    ps_m[m][:, u * 512:(u + 1) * 512],
                                    wqkv_t[:, kk * 384 + m * 128: kk * 384 + (m + 1) * 128],
                                    xs[:, u * 512:(u + 1) * 512],
                                    start=(kk == 0), stop=(kk == 7))
                    cols = slice(tp * 1024, (tp + 1) * 1024)
                    for j in range(HPC):
                        nc.scalar.copy(q8T[b][j][0:64, cols],
                                       ps_m[0][64 * j:64 * (j + 1), :])
                        nc.scalar.copy(kTt[b][j][0:64, cols],
                                       ps_m[1][64 * j:64 * (j + 1), :])
                    nc.scalar.copy(vTt[b][:, cols], ps_m[2][:, :])

            # ---------------- phase 1b: v token-major + ones column ----------------
            v2 = [[y_pool.tile([128, 16 * 65], BF16, tag=f"v2_{b}{j}", name=f"v2_{b}{j}")
                   for j in range(HPC)] for b in range(B)]
            for b in range(B):
                for j in range(HPC):
                    nc.sync.dma_start(v2[b][j][:, 64::65], ones_col[:, :])
                    for kt in range(16):
                        pv = psA.tile([128, 1024], F32, tag="ps")
                        nc.tensor.transpose(
                            pv[:, 0:64],
                            vTt[b][64 * j:64 * (j + 1),
                                   kt * 128:(kt + 1) * 128].bitcast(F32),
                            id64_t[64 * j:64 * (j + 1), :])
                        nc.scalar.copy(v2[b][j][:, kt * 65:kt * 65 + 64], pv[:, 0:64])

            # ---------------- phase 2: attention ----------------
            a2a_in = [dram.tile([8, 128, 128], F32R, tag=f"a2a_in{q_}",
                                name=f"a2a_in{q_}") for q_ in range(NSPAN)]
            a2a_out = [dram.tile([8, 128, 128], F32R, tag=f"a2a_out{q_}",
                                 name=f"a2a_out{q_}") for q_ in range(NSPAN)]

            for Q in range(NSPAN):
                # ---- A-phase: -max(8 q.k) over valid keys -> q8T row 64 ----
                for b in range(B):
                    for j in range(HPC):
                        macc = stats.tile([128, 4], F32, tag=f"macc{b}{j}",
                                          name=f"macc{b}{j}")
                        for ii in range(4):
                            i = 4 * Q + ii
                            nkeys = (i + 1) * 128
                            nchunks = (nkeys + 1023) // 1024
                            for kc in range(nchunks):
                                n = min(1024, nkeys - kc * 1024)
                                pa = psA.tile([128, 1024], F32, tag="ps")
                                for u_ in range(0, n, 512):
                                    nn = min(512, n - u_)
                                    nc.tensor.matmul(
                                        pa[:, u_:u_ + nn],
                                        q8T[b][j][0:64, i * 128:(i + 1) * 128],
                                        kTt[b][j][0:64,
                                                  kc * 1024 + u_:kc * 1024 + u_ + nn],
                                        start=True,
                                        stop=not (kc == nchunks - 1
                                                  and nkeys - 128 >= kc * 1024 + u_
                                                  and nkeys - 128 < kc * 1024 + u_ + nn))
                                if kc == nchunks - 1:
                                    dcol = nkeys - 128 - kc * 1024
                                    nc.tensor.matmul(
                                        pa[:, dcol:dcol + 128],
                                        id16_t[:], maskA_t[:],
                                        start=False, stop=True)
                                if kc == 0:
                                    nc.vector.tensor_reduce(
                                        macc[:, ii:ii + 1], pa[:, 0:n],
                                        axis=mybir.AxisListType.X,
                                        op=mybir.AluOpType.max)
                                else:
                                    mtmp = small.tile([128, 1], F32, tag="mtmp")
                                    nc.vector.tensor_reduce(
                                        mtmp[:], pa[:, 0:n],
                                        axis=mybir.AxisListType.X,
                                        op=mybir.AluOpType.max)
                                    nc.vector.tensor_tensor(
                                        macc[:, ii:ii + 1],
                                        macc[:, ii:ii + 1], mtmp[:],
                                        op=mybir.AluOpType.max)
                        # negate, transpose [128,4]->[4,128], scatter into row 64
                        mneg = stats.tile([128, 4], F32, tag=f"mneg{b}{j}",
                                          name=f"mneg{b}{j}")
                        nc.vector.tensor_scalar(
                            mneg[:], macc[:], -1.0, bmneg_t[:, j:j + 1],
                            op0=mybir.AluOpType.mult, op1=mybir.AluOpType.add)
                        tp = psA.tile([128, 1024], F32, tag="ps")
                        nc.tensor.transpose(tp[0:4, 0:128], mneg[:], idf_t[:])
                        mtr = small.tile([4, 128], F32, tag="mtr")
                        nc.scalar.copy(mtr[:], tp[0:4, 0:128])
                        nc.gpsimd.dma_start(
                            q8T[b][j][64:65, Q * 512:(Q + 1) * 512]
                            .rearrange("o (t p) -> o t p", t=4),
                            mtr[:])

                if Q > 0:
                    nc.gpsimd.collective_compute(
                        "AllToAll", mybir.AluOpType.bypass,
                        replica_groups=[list(range(N_CORES))],
                        ins=[a2a_in[Q - 1].opt()], outs=[a2a_out[Q - 1].opt()])

                # ---- B-phase: scores^T (K=65 folds -mhat), exp, AV ----
                for j in range(HPC):
                    pY = {}
                    for b in range(B):
                        pY[b] = psY.tile([128, 512], F32, tag="psY",
                                         name=f"pY{b}{j}")
                    for kt2 in range(0, 4 * Q + 4, 2):
                        btp = bias_pool.tile([128, 1024], F16, tag="bias",
                                             name="btp")
                        for u_ in range(2):
                            nc.sync.dma_start(
                                btp[:, u_ * 512:(u_ + 1) * 512],
                                biasT[j, (kt2 + u_) * 128:(kt2 + u_ + 1) * 128,
                                      Q * 512:(Q + 1) * 512])
                        for b in range(B):
                            pb = psA.tile([128, 1024], F32, tag="ps")
                            for u in range(2):
                                cols = slice(u * 512, (u + 1) * 512)
                                nc.tensor.matmul(
                                    pb[:, cols],
                                    kTt[b][j][:, (kt2 + u) * 128:(kt2 + u + 1) * 128],
                                    q8T[b][j][:, Q * 512:(Q + 1) * 512],
                                    start=True, stop=True)
                            nc.vector.tensor_tensor(
                                pb[:], pb[:], btp[:], op=mybir.AluOpType.add)
                            pt = p_pool.tile([128, 1024], BF16, tag="p")
                            nc.scalar.activation(pt[:], pb[:], AF.Exp)
                            for u in range(2):
                                nc.tensor.matmul(
                                    pY[b][0:65, :],
                                    v2[b][j][:, (kt2 + u) * 65:(kt2 + u + 1) * 65],
                                    pt[:, u * 512:(u + 1) * 512],
                                    start=(kt2 + u == 0),
                                    stop=(kt2 + u == 4 * Q + 3))

                    # ---- normalize ----
                    for b in range(B):
                        linv = small.tile([1, 512], F32, tag="linv")
                        nc.vector.reciprocal(linv[:], pY[b][64:65, :])
                        linb = small.tile([64, 512], F32, tag="linb")
                        nc.gpsimd.partition_broadcast(linb[:], linv[:], channels=64)
                        ytmp = small.tile([64, 512], F32R, tag="ytmp")
                        nc.vector.tensor_tensor(
                            ytmp[:], pY[b][0:64, :], linb[:],
                            op=mybir.AluOpType.mult)
                        nc.sync.dma_start(
                            a2a_in[Q][:, 64 * j:64 * (j + 1),
                                      64 * b:64 * (b + 1)]
                            .rearrange("r c i -> c r i"),
                            ytmp[:].rearrange("c (r i) -> c r i", r=8))


            # ---------------- phase 4: output projection ----------------
            nc.gpsimd.collective_compute(
                "AllToAll", mybir.AluOpType.bypass,
                replica_groups=[list(range(N_CORES))],
                ins=[a2a_in[NSPAN - 1].opt()], outs=[a2a_out[NSPAN - 1].opt()])
            yin = [yinp.tile([128, 512], F32R, tag=f"yin{r}", name=f"yin{r}") for r in range(8)]
            for r in range(8):
                for q_ in range(NSPAN):
                    nc.sync.dma_start(yin[r][:, q_ * 128:(q_ + 1) * 128],
                                      a2a_out[q_][r])
            for oc in range(2):
                pp = [psA.tile([128, 1024], F32, tag="ps", name=f"pp{h_}")
                      for h_ in range(2)]
                for r in range(8):
                    wp = stream.tile([128, 512], F32R, tag="wp")
                    nc.sync.dma_start(
                        wp[:], wprojT[r * 128:(r + 1) * 128,
                                      oc * 512:(oc + 1) * 512])
                    for tt in range(4):
                        nc.tensor.matmul(
                            pp[tt // 2][:, (tt % 2) * 512:(tt % 2 + 1) * 512],
                            yin[r][:, tt * 128:(tt + 1) * 128],
                            wp[:],
                            start=(r == 0), stop=(r == 7))
                for tt in range(4):
                    ob = stream.tile([128, 512], F32, tag="ob")
                    nc.scalar.copy(ob[:], pp[tt // 2][:, (tt % 2) * 512:(tt % 2 + 1) * 512])
                    nc.sync.dma_start(
                        out[tt * 128:(tt + 1) * 128, oc * 512:(oc + 1) * 512], ob[:])

    nc.finalize()
    return nc


def _prep_inputs(x, position_bias, W_attn, W_proj):
    """Host-side shard/layout prep. Returns in_maps for the 8 cores."""
    x = np.asarray(x, np.float32)
    pb = np.asarray(position_bias, np.float32)[0]          # [H, T, T]
    W_attn = np.asarray(W_attn, np.float32)
    W_proj = np.asarray(W_proj, np.float32)

    xT = np.ascontiguousarray(x.transpose(0, 2, 1))        # [B, C, T]
    wprojT = np.ascontiguousarray(W_proj.T)                # [in, out]
    id_f = np.eye(128, dtype=np.float32)
    maskA = np.triu(np.full((128, 128), NEG, np.float32), 1)  # key>query -> -1e9
    id16 = np.eye(128, dtype=np.float32).astype(ml_dtypes.bfloat16)
    maskA16_np = maskA.astype(ml_dtypes.bfloat16)
    ones_neg = np.full((1, 128), -1.0, np.float32)
    ones_col_np = np.ones((128, 16), ml_dtypes.bfloat16)
    id64x2_np = np.vstack([np.eye(64, dtype=np.float32)] * 2)
    ones_row_np = np.ones((1, T), np.float32)
    esel_np = np.zeros((33, 128), np.float32)
    esel_np[0, 0:64] = 1.0
    esel_np[32, 64:128] = 1.0

    tril = np.tril(np.ones((T, T), dtype=bool))
    in_maps = []
    for c in range(N_CORES):
        wq = W_attn[128 * c:128 * (c + 1), :] * 8.0
        wk = W_attn[C + 128 * c:C + 128 * (c + 1), :]
        wv = W_attn[2 * C + 128 * c:2 * C + 128 * (c + 1), :]
        wqkvT = np.ascontiguousarray(np.concatenate([wq, wk, wv], 0).T)
        bt = np.empty((HPC, T, T), np.float16)
        bm = np.empty((HPC,), np.float32)
        for j in range(HPC):
            h = HPC * c + j
            bh = pb[h]
            bm[j] = -8.0 * float(bh[tril].max())
            btj = (8.0 * bh.T).astype(np.float16)          # [key, query]
            btj[~tril.T] = np.float16(-60000.0)            # key > query
            bt[j] = btj
        in_maps.append({
            "xT": xT, "wqkvT": wqkvT, "biasT": np.ascontiguousarray(bt),
            "wprojT": wprojT, "id_r": id_f, "id_f": id_f,
            "idf16": id_f.astype(np.float16),
            "bmneg": np.broadcast_to(bm, (128, HPC)).copy(),
            "maskA16": maskA16_np, "id16": id16, "id64x2": id64x2_np,
            "ones_neg": ones_neg, "ones_col": ones_col_np,
            "ones_row": ones_row_np, "esel": esel_np,
        })
    return in_maps


def kernel(x, position_bias, W_attn, W_proj, _trace=False, _tmpdir=None):
    if "nc" not in _CACHE:
        _CACHE["nc"] = _build()
    nc = _CACHE["nc"]
    in_maps = _prep_inputs(x, position_bias, W_attn, W_proj)
    res = run_bass_kernel_spmd(nc, in_maps, list(range(N_CORES)),
                               trace=_trace, tmpdir=_tmpdir)
    if _trace:
        _CACHE["exec_time_ns"] = res.exec_time_ns
    out_full = np.empty((B, T, C), np.float32)
    for c in range(N_CORES):
        r = res.results[c]["out"].reshape(NSPAN, B, 64, C)
        for b in range(B):
            for Qs in range(NSPAN):
                out_full[b, Qs * 512 + 64 * c: Qs * 512 + 64 * (c + 1)] = r[Qs, b]
    return out_full


# revision 29
# speedup vs baseline: 1.1996x; 1.0322x over previous
"""Multi-head self-attention (B=2, T=2048, C=1024, H=16, causal, position bias)
on 8 Trainium2 NeuronCores.

Sharding: 2 heads per core (tensor parallel over heads), both batches on every
core. QKV projection computed per-core for its own head slice (x replicated,
pre-transposed on host). Attention fully per-core. Output projection is
token-sharded after an on-device AllToAll of the head-sharded attention
output; host concatenates the 8 token slices.

Numerics: all matmuls fp32r (~13-bit mantissa, full PE rate at N>=256).
The 1/scale = *8 is folded into Wq on the host; per-head causal bias max is
folded into the bias prep so exp() never overflows; causal mask is baked into
the pre-transposed bias tiles as -1e9.
"""
import numpy as np
import ml_dtypes

import concourse.bass as bass
import concourse.mybir as mybir
import concourse.tile as tile
from concourse import bacc
from concourse._compat import get_trn_type
from concourse.bass_utils import run_bass_kernel_spmd

F32 = mybir.dt.float32
F32R = mybir.dt.float32r
BF16 = mybir.dt.bfloat16
F16 = mybir.dt.float16
AF = mybir.ActivationFunctionType

N_CORES = 8
B = 2
T = 2048
C = 1024
H = 16
D = 64
HPC = H // N_CORES        # heads per core = 2
TQ = 128                  # query tile (layout A partitions)
KT = 128                  # key tile (layout B partitions)
QS = 512                  # query span (layout B free dim)
NSPAN = T // QS           # 4 spans per (b, head)
NEG = -1.0e9

_CACHE = {}


def _build():
    nc = bacc.Bacc(get_trn_type() or "TRN2", target_bir_lowering=False,
                   debug=False, num_devices=N_CORES)

    # ---- per-core DRAM parameters (contents differ per core) ----
    xT = nc.declare_dram_parameter("xT", [B, C, T], F32R, isOutput=False)           # x transposed
    wqkvT = nc.declare_dram_parameter("wqkvT", [C, 3 * 128], F32R, isOutput=False)  # [in, q8|k|v]
    biasT = nc.declare_dram_parameter("biasT", [HPC, T, T], F16, isOutput=False)   # masked, *8
    idf16 = nc.declare_dram_parameter("idf16", [128, 128], F16, isOutput=False)
    bmneg = nc.declare_dram_parameter("bmneg", [128, HPC], F32, isOutput=False)  # -8*bmax per head
    wprojT = nc.declare_dram_parameter("wprojT", [C, C], F16, isOutput=False)      # W_proj.T
    id_r = nc.declare_dram_parameter("id_r", [128, 128], F32R, isOutput=False)      # identity
    id_f = nc.declare_dram_parameter("id_f", [128, 128], F32, isOutput=False)       # identity (transpose)
    id64x2 = nc.declare_dram_parameter("id64x2", [128, 64], F32, isOutput=False)  # [I64; I64]
    maskA16 = nc.declare_dram_parameter("maskA16", [128, 128], BF16, isOutput=False)  # strict-upper -1e9
    id16 = nc.declare_dram_parameter("id16", [128, 128], BF16, isOutput=False)
    ones_neg = nc.declare_dram_parameter("ones_neg", [1, 128], F32R, isOutput=False)  # all -1.0
    ones_col = nc.declare_dram_parameter("ones_col", [128, 16], BF16, isOutput=False)  # all 1.0
    ones_row = nc.declare_dram_parameter("ones_row", [1, T], F32R, isOutput=False)
    esel = nc.declare_dram_parameter("esel", [33, 128], F32R, isOutput=False)
    out = nc.declare_dram_parameter("out", [T * B // N_CORES, C], F32, isOutput=True)

    with tile.TileContext(nc) as tc:
        with (
            tc.tile_pool(name="consts", bufs=1) as consts,
            tc.tile_pool(name="wq", bufs=1) as wq_pool,
            tc.tile_pool(name="qkv", bufs=1) as qkv_pool,
            tc.tile_pool(name="stream", bufs=3) as stream,
            tc.tile_pool(name="bias", bufs=8) as bias_pool,
            tc.tile_pool(name="ptile", bufs=4) as p_pool,
            tc.tile_pool(name="yinp", bufs=1) as yinp,
            tc.tile_pool(name="stats", bufs=1) as stats,
            tc.tile_pool(name="ytile", bufs=1) as y_pool,
            tc.tile_pool(name="small", bufs=2) as small,
            tc.tile_pool(name="psA", bufs=3, space="PSUM") as psA,
            tc.tile_pool(name="psY", bufs=2, space="PSUM") as psY,
            tc.tile_pool(name="dram", bufs=1, space="DRAM") as dram,
        ):
            # ---------------- constants ----------------
            idr_t = consts.tile([128, 128], F32R, tag="idr")
            nc.sync.dma_start(idr_t[:], id_r[:])
            idf_t = consts.tile([128, 128], F32, tag="idf")
            nc.sync.dma_start(idf_t[:], id_f[:])
            maskA_t = consts.tile([128, 128], BF16, tag="maskA")
            nc.sync.dma_start(maskA_t[:], maskA16[:])
            id16_t = consts.tile([128, 128], BF16, tag="id16")
            nc.sync.dma_start(id16_t[:], id16[:])
            onesneg_t = consts.tile([1, 128], F32R, tag="onesneg")
            nc.sync.dma_start(onesneg_t[:], ones_neg[:])
            id64_t = consts.tile([128, 64], F32, tag="id64")
            nc.sync.dma_start(id64_t[:], id64x2[:])
            esel_t = consts.tile([33, 128], F32R, tag="esel")
            nc.sync.dma_start(esel_t[:], esel[:])
            idf16_t = consts.tile([128, 128], F16, tag="idf16")
            nc.sync.dma_start(idf16_t[:], idf16[:])
            bmneg_t = consts.tile([128, HPC], F32, tag="bmneg")
            nc.sync.dma_start(bmneg_t[:], bmneg[:])

            wqkv_t = wq_pool.tile([128, 8 * 384], F32R, tag="wqkv")
            for kk in range(8):
                nc.sync.dma_start(wqkv_t[:, kk * 384:(kk + 1) * 384],
                                  wqkvT[kk * 128:(kk + 1) * 128, :])

            # ---------------- phase 1: QKV projection ----------------
            # q8T/kT per (b, head): [65, 2048]; row 64: q8T = -mhat (per span),
            # kT = 1.0 (host). vT per b: [128 (2 heads), 2048].
            q8T = [[qkv_pool.tile([65, T], F32R, tag=f"q8T{b}{j}", name=f"q8T{b}{j}")
                    for j in range(HPC)] for b in range(B)]
            kTt = [[qkv_pool.tile([65, T], F32R, tag=f"kT{b}{j}", name=f"kT{b}{j}")
                    for j in range(HPC)] for b in range(B)]
            vTt = [qkv_pool.tile([128, T], F32R, tag=f"vT{b}", name=f"vT{b}")
                   for b in range(B)]
            for b in range(B):
                for j in range(HPC):
                    nc.sync.dma_start(kTt[b][j][64:65, :], ones_row[:, :])
            for b in range(B):
                for tp in range(2):
                    ps_m = [psA.tile([128, 1024], F32, tag="ps", name=f"psm{m_}")
                            for m_ in range(3)]
                    for kk in range(8):
                        xs = stream.tile([128, 1024], F32R, tag="xs")
                        nc.sync.dma_start(
                            xs[:], xT[b, kk * 128:(kk + 1) * 128,
                                      tp * 1024:(tp + 1) * 1024])
                        for m in range(3):
                            for u in range(2):
                                nc.tensor.matmul(
                                    ps_m[m][:, u * 512:(u + 1) * 512],
                                    wqkv_t[:, kk * 384 + m * 128: kk * 384 + (m + 1) * 128],
                                    xs[:, u * 512:(u + 1) * 512],
                                    start=(kk == 0), stop=(kk == 7))
                    cols = slice(tp * 1024, (tp + 1) * 1024)
                    for j in range(HPC):
                        nc.scalar.copy(q8T[b][j][0:64, cols],
                                       ps_m[0][64 * j:64 * (j + 1), :])
                        nc.scalar.copy(kTt[b][j][0:64, cols],
                                       ps_m[1][64 * j:64 * (j + 1), :])
                    nc.scalar.copy(vTt[b][:, cols], ps_m[2][:, :])

            # ---------------- phase 1b: v token-major + ones column ----------------
            v2 = [[y_pool.tile([128, 16 * 65], BF16, tag=f"v2_{b}{j}", name=f"v2_{b}{j}")
                   for j in range(HPC)] for b in range(B)]
            for b in range(B):
                for j in range(HPC):
                    nc.sync.dma_start(v2[b][j][:, 64::65], ones_col[:, :])
                    for kt in range(16):
                        pv = psA.tile([128, 1024], F32, tag="ps")
                        nc.tensor.transpose(
                            pv[:, 0:64],
                            vTt[b][64 * j:64 * (j + 1),
                                   kt * 128:(kt + 1) * 128].bitcast(F32),
                            id64_t[64 * j:64 * (j + 1), :])
                        nc.scalar.copy(v2[b][j][:, kt * 65:kt * 65 + 64], pv[:, 0:64])

            # ---------------- phase 2: attention ----------------
            a2a_in = [dram.tile([8, 128, 128], F16, tag=f"a2a_in{q_}",
                                name=f"a2a_in{q_}") for q_ in range(NSPAN)]
            a2a_out = [dram.tile([8, 128, 128], F16, tag=f"a2a_out{q_}",
                                 name=f"a2a_out{q_}") for q_ in range(NSPAN)]

            for Q in range(NSPAN):
                # ---- A-phase: -max(8 q.k) over valid keys -> q8T row 64 ----
                for b in range(B):
                    for j in range(HPC):
                        macc = stats.tile([128, 4], F32, tag=f"macc{b}{j}",
                                          name=f"macc{b}{j}")
                        for ii in range(4):
                            i = 4 * Q + ii
                            nkeys = (i + 1) * 128
                            nchunks = (nkeys + 1023) // 1024
                            for kc in range(nchunks):
                                n = min(1024, nkeys - kc * 1024)
                                pa = psA.tile([128, 1024], F32, tag="ps")
                                for u_ in range(0, n, 512):
                                    nn = min(512, n - u_)
                                    nc.tensor.matmul(
                                        pa[:, u_:u_ + nn],
                                        q8T[b][j][0:64, i * 128:(i + 1) * 128],
                                        kTt[b][j][0:64,
                                                  kc * 1024 + u_:kc * 1024 + u_ + nn],
                                        start=True,
                                        stop=not (kc == nchunks - 1
                                                  and nkeys - 128 >= kc * 1024 + u_
                                                  and nkeys - 128 < kc * 1024 + u_ + nn))
                                if kc == nchunks - 1:
                                    dcol = nkeys - 128 - kc * 1024
                                    nc.tensor.matmul(
                                        pa[:, dcol:dcol + 128],
                                        id16_t[:], maskA_t[:],
                                        start=False, stop=True)
                                if kc == 0:
                                    nc.vector.tensor_reduce(
                                        macc[:, ii:ii + 1], pa[:, 0:n],
                                        axis=mybir.AxisListType.X,
                                        op=mybir.AluOpType.max)
                                else:
                                    mtmp = small.tile([128, 1], F32, tag="mtmp")
                                    nc.vector.tensor_reduce(
                                        mtmp[:], pa[:, 0:n],
                                        axis=mybir.AxisListType.X,
                                        op=mybir.AluOpType.max)
                                    nc.vector.tensor_tensor(
                                        macc[:, ii:ii + 1],
                                        macc[:, ii:ii + 1], mtmp[:],
                                        op=mybir.AluOpType.max)
                        # negate, transpose [128,4]->[4,128], scatter into row 64
                        mneg = stats.tile([128, 4], F32, tag=f"mneg{b}{j}",
                                          name=f"mneg{b}{j}")
                        nc.vector.tensor_scalar(
                            mneg[:], macc[:], -1.0, bmneg_t[:, j:j + 1],
                            op0=mybir.AluOpType.mult, op1=mybir.AluOpType.add)
                        tp = psA.tile([128, 1024], F32, tag="ps")
                        nc.tensor.transpose(tp[0:4, 0:128], mneg[:], idf_t[:])
                        mtr = small.tile([4, 128], F32, tag="mtr")
                        nc.scalar.copy(mtr[:], tp[0:4, 0:128])
                        nc.gpsimd.dma_start(
                            q8T[b][j][64:65, Q * 512:(Q + 1) * 512]
                            .rearrange("o (t p) -> o t p", t=4),
                            mtr[:])

                if Q > 0:
                    nc.gpsimd.collective_compute(
                        "AllToAll", mybir.AluOpType.bypass,
                        replica_groups=[list(range(N_CORES))],
                        ins=[a2a_in[Q - 1].opt()], outs=[a2a_out[Q - 1].opt()])

                # ---- B-phase: scores^T (K=65 folds -mhat), exp, AV ----
                for j in range(HPC):
                    pY = {}
                    for b in range(B):
                        pY[b] = psY.tile([128, 512], F32, tag="psY",
                                         name=f"pY{b}{j}")
                    for kt2 in range(0, 4 * Q + 4, 2):
                        btp = bias_pool.tile([128, 1024], F16, tag="bias",
                                             name="btp")
                        for u_ in range(2):
                            nc.sync.dma_start(
                                btp[:, u_ * 512:(u_ + 1) * 512],
                                biasT[j, (kt2 + u_) * 128:(kt2 + u_ + 1) * 128,
                                      Q * 512:(Q + 1) * 512])
                        for b in range(B):
                            pb = psA.tile([128, 1024], F32, tag="ps")
                            for u in range(2):
                                cols = slice(u * 512, (u + 1) * 512)
                                nc.tensor.matmul(
                                    pb[:, cols],
                                    kTt[b][j][:, (kt2 + u) * 128:(kt2 + u + 1) * 128],
                                    q8T[b][j][:, Q * 512:(Q + 1) * 512],
                                    start=True, stop=True)
                            nc.vector.tensor_tensor(
                                pb[:], pb[:], btp[:], op=mybir.AluOpType.add)
                            pt = p_pool.tile([128, 1024], BF16, tag="p")
                            nc.scalar.activation(pt[:], pb[:], AF.Exp)
                            for u in range(2):
                                nc.tensor.matmul(
                                    pY[b][0:65, :],
                                    v2[b][j][:, (kt2 + u) * 65:(kt2 + u + 1) * 65],
                                    pt[:, u * 512:(u + 1) * 512],
                                    start=(kt2 + u == 0),
                                    stop=(kt2 + u == 4 * Q + 3))

                    # ---- normalize ----
                    for b in range(B):
                        linv = small.tile([1, 512], F32, tag="linv")
                        nc.vector.reciprocal(linv[:], pY[b][64:65, :])
                        linb = small.tile([64, 512], F32, tag="linb")
                        nc.gpsimd.partition_broadcast(linb[:], linv[:], channels=64)
                        ytmp = small.tile([64, 512], F16, tag="ytmp")
                        nc.vector.tensor_tensor(
                            ytmp[:], pY[b][0:64, :], linb[:],
                            op=mybir.AluOpType.mult)
                        nc.sync.dma_start(
                            a2a_in[Q][:, 64 * j:64 * (j + 1),
                                      64 * b:64 * (b + 1)]
                            .rearrange("r c i -> c r i"),
                            ytmp[:].rearrange("c (r i) -> c r i", r=8))


            # ---------------- phase 4: output projection ----------------
            nc.gpsimd.collective_compute(
                "AllToAll", mybir.AluOpType.bypass,
                replica_groups=[list(range(N_CORES))],
                ins=[a2a_in[NSPAN - 1].opt()], outs=[a2a_out[NSPAN - 1].opt()])
            yin = [yinp.tile([128, 512], F16, tag=f"yin{r}", name=f"yin{r}") for r in range(8)]
            for r in range(8):
                for q_ in range(NSPAN):
                    nc.sync.dma_start(yin[r][:, q_ * 128:(q_ + 1) * 128],
                                      a2a_out[q_][r])
            for oc in range(2):
                pp = [psA.tile([128, 1024], F32, tag="ps", name=f"pp{h_}")
                      for h_ in range(2)]
                for r in range(8):
                    wp = stream.tile([128, 512], F16, tag="wp")
                    nc.sync.dma_start(
                        wp[:], wprojT[r * 128:(r + 1) * 128,
                                      oc * 512:(oc + 1) * 512])
                    for tt in range(4):
                        nc.tensor.matmul(
                            pp[tt // 2][:, (tt % 2) * 512:(tt % 2 + 1) * 512],
                            yin[r][:, tt * 128:(tt + 1) * 128],
                            wp[:],
                            start=(r == 0), stop=(r == 7))
                for tt in range(4):
                    ob = stream.tile([128, 512], F32, tag="ob")
                    nc.scalar.copy(ob[:], pp[tt // 2][:, (tt % 2) * 512:(tt % 2 + 1) * 512])
                    nc.sync.dma_start(
                        out[tt * 128:(tt + 1) * 128, oc * 512:(oc + 1) * 512], ob[:])

    nc.finalize()
    return nc


def _prep_inputs(x, position_bias, W_attn, W_proj):
    """Host-side shard/layout prep. Returns in_maps for the 8 cores."""
    x = np.asarray(x, np.float32)
    pb = np.asarray(position_bias, np.float32)[0]          # [H, T, T]
    W_attn = np.asarray(W_attn, np.float32)
    W_proj = np.asarray(W_proj, np.float32)

    xT = np.ascontiguousarray(x.transpose(0, 2, 1))        # [B, C, T]
    wprojT = np.ascontiguousarray(W_proj.T).astype(np.float16)     # [in, out]
    id_f = np.eye(128, dtype=np.float32)
    maskA = np.triu(np.full((128, 128), NEG, np.float32), 1)  # key>query -> -1e9
    id16 = np.eye(128, dtype=np.float32).astype(ml_dtypes.bfloat16)
    maskA16_np = maskA.astype(ml_dtypes.bfloat16)
    ones_neg = np.full((1, 128), -1.0, np.float32)
    ones_col_np = np.ones((128, 16), ml_dtypes.bfloat16)
    id64x2_np = np.vstack([np.eye(64, dtype=np.float32)] * 2)
    ones_row_np = np.ones((1, T), np.float32)
    esel_np = np.zeros((33, 128), np.float32)
    esel_np[0, 0:64] = 1.0
    esel_np[32, 64:128] = 1.0

    tril = np.tril(np.ones((T, T), dtype=bool))
    in_maps = []
    for c in range(N_CORES):
        wq = W_attn[128 * c:128 * (c + 1), :] * 8.0
        wk = W_attn[C + 128 * c:C + 128 * (c + 1), :]
        wv = W_attn[2 * C + 128 * c:2 * C + 128 * (c + 1), :]
        wqkvT = np.ascontiguousarray(np.concatenate([wq, wk, wv], 0).T)
        bt = np.empty((HPC, T, T), np.float16)
        bm = np.empty((HPC,), np.float32)
        for j in range(HPC):
            h = HPC * c + j
            bh = pb[h]
            bm[j] = -8.0 * float(bh[tril].max())
            btj = (8.0 * bh.T).astype(np.float16)          # [key, query]
            btj[~tril.T] = np.float16(-60000.0)            # key > query
            bt[j] = btj
        in_maps.append({
            "xT": xT, "wqkvT": wqkvT, "biasT": np.ascontiguousarray(bt),
            "wprojT": wprojT, "id_r": id_f, "id_f": id_f,
            "idf16": id_f.astype(np.float16),
            "bmneg": np.broadcast_to(bm, (128, HPC)).copy(),
            "maskA16": maskA16_np, "id16": id16, "id64x2": id64x2_np,
            "ones_neg": ones_neg, "ones_col": ones_col_np,
            "ones_row": ones_row_np, "esel": esel_np,
        })
    return in_maps


def kernel(x, position_bias, W_attn, W_proj, _trace=False, _tmpdir=None):
    if "nc" not in _CACHE:
        _CACHE["nc"] = _build()
    nc = _CACHE["nc"]
    in_maps = _prep_inputs(x, position_bias, W_attn, W_proj)
    res = run_bass_kernel_spmd(nc, in_maps, list(range(N_CORES)),
                               trace=_trace, tmpdir=_tmpdir)
    if _trace:
        _CACHE["exec_time_ns"] = res.exec_time_ns
    out_full = np.empty((B, T, C), np.float32)
    for c in range(N_CORES):
        r = res.results[c]["out"].reshape(NSPAN, B, 64, C)
        for b in range(B):
            for Qs in range(NSPAN):
                out_full[b, Qs * 512 + 64 * c: Qs * 512 + 64 * (c + 1)] = r[Qs, b]
    return out_full


# revision 30
# speedup vs baseline: 1.2053x; 1.0048x over previous
"""Multi-head self-attention (B=2, T=2048, C=1024, H=16, causal, position bias)
on 8 Trainium2 NeuronCores.

Sharding: 2 heads per core (tensor parallel over heads), both batches on every
core. QKV projection computed per-core for its own head slice (x replicated,
pre-transposed on host). Attention fully per-core. Output projection is
token-sharded after an on-device AllToAll of the head-sharded attention
output; host concatenates the 8 token slices.

Numerics: all matmuls fp32r (~13-bit mantissa, full PE rate at N>=256).
The 1/scale = *8 is folded into Wq on the host; per-head causal bias max is
folded into the bias prep so exp() never overflows; causal mask is baked into
the pre-transposed bias tiles as -1e9.
"""
import numpy as np
import ml_dtypes

import concourse.bass as bass
import concourse.mybir as mybir
import concourse.tile as tile
from concourse import bacc
from concourse._compat import get_trn_type
from concourse.bass_utils import run_bass_kernel_spmd

F32 = mybir.dt.float32
F32R = mybir.dt.float32r
BF16 = mybir.dt.bfloat16
F16 = mybir.dt.float16
AF = mybir.ActivationFunctionType

N_CORES = 8
B = 2
T = 2048
C = 1024
H = 16
D = 64
HPC = H // N_CORES        # heads per core = 2
TQ = 128                  # query tile (layout A partitions)
KT = 128                  # key tile (layout B partitions)
QS = 512                  # query span (layout B free dim)
NSPAN = T // QS           # 4 spans per (b, head)
NEG = -1.0e9

_CACHE = {}


def _build():
    nc = bacc.Bacc(get_trn_type() or "TRN2", target_bir_lowering=False,
                   debug=False, num_devices=N_CORES)

    # ---- per-core DRAM parameters (contents differ per core) ----
    xT = nc.declare_dram_parameter("xT", [B, C, T], F32R, isOutput=False)           # x transposed
    wqkvT = nc.declare_dram_parameter("wqkvT", [C, 3 * 128], F32R, isOutput=False)  # [in, q8|k|v]
    biasT = nc.declare_dram_parameter("biasT", [HPC, T, T], F16, isOutput=False)   # masked, *8
    idf16 = nc.declare_dram_parameter("idf16", [128, 128], F16, isOutput=False)
    bmneg = nc.declare_dram_parameter("bmneg", [128, HPC], F32, isOutput=False)  # -8*bmax per head
    wprojT = nc.declare_dram_parameter("wprojT", [C, C], F16, isOutput=False)      # W_proj.T
    id_r = nc.declare_dram_parameter("id_r", [128, 128], F32R, isOutput=False)      # identity
    id_f = nc.declare_dram_parameter("id_f", [128, 128], F32, isOutput=False)       # identity (transpose)
    id64x2 = nc.declare_dram_parameter("id64x2", [128, 64], F32, isOutput=False)  # [I64; I64]
    maskA16 = nc.declare_dram_parameter("maskA16", [128, 128], BF16, isOutput=False)  # strict-upper -1e9
    id16 = nc.declare_dram_parameter("id16", [128, 128], BF16, isOutput=False)
    ones_neg = nc.declare_dram_parameter("ones_neg", [1, 128], F32R, isOutput=False)  # all -1.0
    ones_col = nc.declare_dram_parameter("ones_col", [128, 16], BF16, isOutput=False)  # all 1.0
    ones_row = nc.declare_dram_parameter("ones_row", [1, T], F32R, isOutput=False)
    esel = nc.declare_dram_parameter("esel", [33, 128], F32R, isOutput=False)
    out = nc.declare_dram_parameter("out", [T * B // N_CORES, C], F32, isOutput=True)

    with tile.TileContext(nc) as tc:
        with (
            tc.tile_pool(name="consts", bufs=1) as consts,
            tc.tile_pool(name="wq", bufs=1) as wq_pool,
            tc.tile_pool(name="qkv", bufs=1) as qkv_pool,
            tc.tile_pool(name="stream", bufs=3) as stream,
            tc.tile_pool(name="bias", bufs=8) as bias_pool,
            tc.tile_pool(name="ptile", bufs=4) as p_pool,
            tc.tile_pool(name="yinp", bufs=1) as yinp,
            tc.tile_pool(name="stats", bufs=1) as stats,
            tc.tile_pool(name="ytile", bufs=1) as y_pool,
            tc.tile_pool(name="small", bufs=2) as small,
            tc.tile_pool(name="psA", bufs=3, space="PSUM") as psA,
            tc.tile_pool(name="psY", bufs=2, space="PSUM") as psY,
            tc.tile_pool(name="dram", bufs=1, space="DRAM") as dram,
        ):
            # ---------------- constants ----------------
            idr_t = consts.tile([128, 128], F32R, tag="idr")
            nc.sync.dma_start(idr_t[:], id_r[:])
            idf_t = consts.tile([128, 128], F32, tag="idf")
            nc.sync.dma_start(idf_t[:], id_f[:])
            maskA_t = consts.tile([128, 128], BF16, tag="maskA")
            nc.sync.dma_start(maskA_t[:], maskA16[:])
            id16_t = consts.tile([128, 128], BF16, tag="id16")
            nc.sync.dma_start(id16_t[:], id16[:])
            onesneg_t = consts.tile([1, 128], F32R, tag="onesneg")
            nc.sync.dma_start(onesneg_t[:], ones_neg[:])
            id64_t = consts.tile([128, 64], F32, tag="id64")
            nc.sync.dma_start(id64_t[:], id64x2[:])
            esel_t = consts.tile([33, 128], F32R, tag="esel")
            nc.sync.dma_start(esel_t[:], esel[:])
            idf16_t = consts.tile([128, 128], F16, tag="idf16")
            nc.sync.dma_start(idf16_t[:], idf16[:])
            bmneg_t = consts.tile([128, HPC], F32, tag="bmneg")
            nc.sync.dma_start(bmneg_t[:], bmneg[:])

            wproj_t = wq_pool.tile([128, 8 * 1024], F16, tag="wproj")
            for r in range(8):
                nc.sync.dma_start(wproj_t[:, r * 1024:(r + 1) * 1024],
                                  wprojT[r * 128:(r + 1) * 128, :])
            wqkv_t = wq_pool.tile([128, 8 * 384], F32R, tag="wqkv")
            for kk in range(8):
                nc.sync.dma_start(wqkv_t[:, kk * 384:(kk + 1) * 384],
                                  wqkvT[kk * 128:(kk + 1) * 128, :])

            # ---------------- phase 1: QKV projection ----------------
            # q8T/kT per (b, head): [65, 2048]; row 64: q8T = -mhat (per span),
            # kT = 1.0 (host). vT per b: [128 (2 heads), 2048].
            q8T = [[qkv_pool.tile([65, T], F32R, tag=f"q8T{b}{j}", name=f"q8T{b}{j}")
                    for j in range(HPC)] for b in range(B)]
            kTt = [[qkv_pool.tile([65, T], F32R, tag=f"kT{b}{j}", name=f"kT{b}{j}")
                    for j in range(HPC)] for b in range(B)]
            vTt = [qkv_pool.tile([128, T], F32R, tag=f"vT{b}", name=f"vT{b}")
                   for b in range(B)]
            for b in range(B):
                for j in range(HPC):
                    nc.sync.dma_start(kTt[b][j][64:65, :], ones_row[:, :])
            for b in range(B):
                for tp in range(2):
                    ps_m = [psA.tile([128, 1024], F32, tag="ps", name=f"psm{m_}")
                            for m_ in range(3)]
                    for kk in range(8):
                        xs = stream.tile([128, 1024], F32R, tag="xs")
                        nc.sync.dma_start(
                            xs[:], xT[b, kk * 128:(kk + 1) * 128,
                                      tp * 1024:(tp + 1) * 1024])
                        for m in range(3):
                            for u in range(2):
                                nc.tensor.matmul(
                                    ps_m[m][:, u * 512:(u + 1) * 512],
                                    wqkv_t[:, kk * 384 + m * 128: kk * 384 + (m + 1) * 128],
                                    xs[:, u * 512:(u + 1) * 512],
                                    start=(kk == 0), stop=(kk == 7))
                    cols = slice(tp * 1024, (tp + 1) * 1024)
                    for j in range(HPC):
                        nc.scalar.copy(q8T[b][j][0:64, cols],
                                       ps_m[0][64 * j:64 * (j + 1), :])
                        nc.scalar.copy(kTt[b][j][0:64, cols],
                                       ps_m[1][64 * j:64 * (j + 1), :])
                    nc.scalar.copy(vTt[b][:, cols], ps_m[2][:, :])

            # ---------------- phase 1b: v token-major + ones column ----------------
            v2 = [[y_pool.tile([128, 16 * 65], BF16, tag=f"v2_{b}{j}", name=f"v2_{b}{j}")
                   for j in range(HPC)] for b in range(B)]
            for b in range(B):
                for j in range(HPC):
                    nc.sync.dma_start(v2[b][j][:, 64::65], ones_col[:, :])
                    for kt in range(16):
                        pv = psA.tile([128, 1024], F32, tag="ps")
                        nc.tensor.transpose(
                            pv[:, 0:64],
                            vTt[b][64 * j:64 * (j + 1),
                                   kt * 128:(kt + 1) * 128].bitcast(F32),
                            id64_t[64 * j:64 * (j + 1), :])
                        nc.scalar.copy(v2[b][j][:, kt * 65:kt * 65 + 64], pv[:, 0:64])

            # ---------------- phase 2: attention ----------------
            a2a_in = [dram.tile([8, 128, 128], F16, tag=f"a2a_in{q_}",
                                name=f"a2a_in{q_}") for q_ in range(NSPAN)]
            a2a_out = [dram.tile([8, 128, 128], F16, tag=f"a2a_out{q_}",
                                 name=f"a2a_out{q_}") for q_ in range(NSPAN)]
            yin = [yinp.tile([128, 512], F16, tag=f"yin{r}", name=f"yin{r}")
                   for r in range(8)]

            def proj_pass(tt):
                for r in range(8):
                    nc.sync.dma_start(yin[r][:, tt * 128:(tt + 1) * 128],
                                      a2a_out[tt][r])
                pp = psA.tile([128, 1024], F32, tag="ps", name=f"pp{tt}")
                for oc in range(2):
                    for r in range(8):
                        nc.tensor.matmul(
                            pp[:, oc * 512:(oc + 1) * 512],
                            yin[r][:, tt * 128:(tt + 1) * 128],
                            wproj_t[:, r * 1024 + oc * 512: r * 1024 + (oc + 1) * 512],
                            start=(r == 0), stop=(r == 7))
                ob = stream.tile([128, 1024], F32, tag="ob")
                nc.scalar.copy(ob[:], pp[:])
                nc.sync.dma_start(out[tt * 128:(tt + 1) * 128, :], ob[:])

            for Q in range(NSPAN):
                # ---- A-phase: -max(8 q.k) over valid keys -> q8T row 64 ----
                for b in range(B):
                    for j in range(HPC):
                        macc = stats.tile([128, 4], F32, tag=f"macc{b}{j}",
                                          name=f"macc{b}{j}")
                        for ii in range(4):
                            i = 4 * Q + ii
                            nkeys = (i + 1) * 128
                            nchunks = (nkeys + 1023) // 1024
                            for kc in range(nchunks):
                                n = min(1024, nkeys - kc * 1024)
                                pa = psA.tile([128, 1024], F32, tag="ps")
                                for u_ in range(0, n, 512):
                                    nn = min(512, n - u_)
                                    nc.tensor.matmul(
                                        pa[:, u_:u_ + nn],
                                        q8T[b][j][0:64, i * 128:(i + 1) * 128],
                                        kTt[b][j][0:64,
                                                  kc * 1024 + u_:kc * 1024 + u_ + nn],
                                        start=True,
                                        stop=not (kc == nchunks - 1
                                                  and nkeys - 128 >= kc * 1024 + u_
                                                  and nkeys - 128 < kc * 1024 + u_ + nn))
                                if kc == nchunks - 1:
                                    dcol = nkeys - 128 - kc * 1024
                                    nc.tensor.matmul(
                                        pa[:, dcol:dcol + 128],
                                        id16_t[:], maskA_t[:],
                                        start=False, stop=True)
                                if kc == 0:
                                    nc.vector.tensor_reduce(
                                        macc[:, ii:ii + 1], pa[:, 0:n],
                                        axis=mybir.AxisListType.X,
                                        op=mybir.AluOpType.max)
                                else:
                                    mtmp = small.tile([128, 1], F32, tag="mtmp")
                                    nc.vector.tensor_reduce(
                                        mtmp[:], pa[:, 0:n],
                                        axis=mybir.AxisListType.X,
                                        op=mybir.AluOpType.max)
                                    nc.vector.tensor_tensor(
                                        macc[:, ii:ii + 1],
                                        macc[:, ii:ii + 1], mtmp[:],
                                        op=mybir.AluOpType.max)
                        # negate, transpose [128,4]->[4,128], scatter into row 64
                        mneg = stats.tile([128, 4], F32, tag=f"mneg{b}{j}",
                                          name=f"mneg{b}{j}")
                        nc.vector.tensor_scalar(
                            mneg[:], macc[:], -1.0, bmneg_t[:, j:j + 1],
                            op0=mybir.AluOpType.mult, op1=mybir.AluOpType.add)
                        tp = psA.tile([128, 1024], F32, tag="ps")
                        nc.tensor.transpose(tp[0:4, 0:128], mneg[:], idf_t[:])
                        mtr = small.tile([4, 128], F32, tag="mtr")
                        nc.scalar.copy(mtr[:], tp[0:4, 0:128])
                        nc.gpsimd.dma_start(
                            q8T[b][j][64:65, Q * 512:(Q + 1) * 512]
                            .rearrange("o (t p) -> o t p", t=4),
                            mtr[:])

                if Q > 0:
                    nc.gpsimd.collective_compute(
                        "AllToAll", mybir.AluOpType.bypass,
                        replica_groups=[list(range(N_CORES))],
                        ins=[a2a_in[Q - 1].opt()], outs=[a2a_out[Q - 1].opt()])
                    proj_pass(Q - 1)

                # ---- B-phase: scores^T (K=65 folds -mhat), exp, AV ----
                for j in range(HPC):
                    pY = {}
                    for b in range(B):
                        pY[b] = psY.tile([128, 512], F32, tag="psY",
                                         name=f"pY{b}{j}")
                    for kt2 in range(0, 4 * Q + 4, 2):
                        btp = bias_pool.tile([128, 1024], F16, tag="bias",
                                             name="btp")
                        for u_ in range(2):
                            nc.sync.dma_start(
                                btp[:, u_ * 512:(u_ + 1) * 512],
                                biasT[j, (kt2 + u_) * 128:(kt2 + u_ + 1) * 128,
                                      Q * 512:(Q + 1) * 512])
                        for b in range(B):
                            pb = psA.tile([128, 1024], F32, tag="ps")
                            for u in range(2):
                                cols = slice(u * 512, (u + 1) * 512)
                                nc.tensor.matmul(
                                    pb[:, cols],
                                    kTt[b][j][:, (kt2 + u) * 128:(kt2 + u + 1) * 128],
                                    q8T[b][j][:, Q * 512:(Q + 1) * 512],
                                    start=True, stop=True)
                            nc.vector.tensor_tensor(
                                pb[:], pb[:], btp[:], op=mybir.AluOpType.add)
                            pt = p_pool.tile([128, 1024], BF16, tag="p")
                            nc.scalar.activation(pt[:], pb[:], AF.Exp)
                            for u in range(2):
                                nc.tensor.matmul(
                                    pY[b][0:65, :],
                                    v2[b][j][:, (kt2 + u) * 65:(kt2 + u + 1) * 65],
                                    pt[:, u * 512:(u + 1) * 512],
                                    start=(kt2 + u == 0),
                                    stop=(kt2 + u == 4 * Q + 3))

                    # ---- normalize ----
                    for b in range(B):
                        linv = small.tile([1, 512], F32, tag="linv")
                        nc.vector.reciprocal(linv[:], pY[b][64:65, :])
                        linb = small.tile([64, 512], F32, tag="linb")
                        nc.gpsimd.partition_broadcast(linb[:], linv[:], channels=64)
                        ytmp = small.tile([64, 512], F16, tag="ytmp")
                        nc.vector.tensor_tensor(
                            ytmp[:], pY[b][0:64, :], linb[:],
                            op=mybir.AluOpType.mult)
                        nc.sync.dma_start(
                            a2a_in[Q][:, 64 * j:64 * (j + 1),
                                      64 * b:64 * (b + 1)]
                            .rearrange("r c i -> c r i"),
                            ytmp[:].rearrange("c (r i) -> c r i", r=8))


            # ---------------- phase 4: final slice ----------------
            nc.gpsimd.collective_compute(
                "AllToAll", mybir.AluOpType.bypass,
                replica_groups=[list(range(N_CORES))],
                ins=[a2a_in[NSPAN - 1].opt()], outs=[a2a_out[NSPAN - 1].opt()])
            proj_pass(NSPAN - 1)

    nc.finalize()
    return nc


def _prep_inputs(x, position_bias, W_attn, W_proj):
    """Host-side shard/layout prep. Returns in_maps for the 8 cores."""
    x = np.asarray(x, np.float32)
    pb = np.asarray(position_bias, np.float32)[0]          # [H, T, T]
    W_attn = np.asarray(W_attn, np.float32)
    W_proj = np.asarray(W_proj, np.float32)

    xT = np.ascontiguousarray(x.transpose(0, 2, 1))        # [B, C, T]
    wprojT = np.ascontiguousarray(W_proj.T).astype(np.float16)     # [in, out]
    id_f = np.eye(128, dtype=np.float32)
    maskA = np.triu(np.full((128, 128), NEG, np.float32), 1)  # key>query -> -1e9
    id16 = np.eye(128, dtype=np.float32).astype(ml_dtypes.bfloat16)
    maskA16_np = maskA.astype(ml_dtypes.bfloat16)
    ones_neg = np.full((1, 128), -1.0, np.float32)
    ones_col_np = np.ones((128, 16), ml_dtypes.bfloat16)
    id64x2_np = np.vstack([np.eye(64, dtype=np.float32)] * 2)
    ones_row_np = np.ones((1, T), np.float32)
    esel_np = np.zeros((33, 128), np.float32)
    esel_np[0, 0:64] = 1.0
    esel_np[32, 64:128] = 1.0

    tril = np.tril(np.ones((T, T), dtype=bool))
    in_maps = []
    for c in range(N_CORES):
        wq = W_attn[128 * c:128 * (c + 1), :] * 8.0
        wk = W_attn[C + 128 * c:C + 128 * (c + 1), :]
        wv = W_attn[2 * C + 128 * c:2 * C + 128 * (c + 1), :]
        wqkvT = np.ascontiguousarray(np.concatenate([wq, wk, wv], 0).T)
        bt = np.empty((HPC, T, T), np.float16)
        bm = np.empty((HPC,), np.float32)
        for j in range(HPC):
            h = HPC * c + j
            bh = pb[h]
            bm[j] = -8.0 * float(bh[tril].max())
            btj = (8.0 * bh.T).astype(np.float16)          # [key, query]
            btj[~tril.T] = np.float16(-60000.0)            # key > query
            bt[j] = btj
        in_maps.append({
            "xT": xT, "wqkvT": wqkvT, "biasT": np.ascontiguousarray(bt),
            "wprojT": wprojT, "id_r": id_f, "id_f": id_f,
            "idf16": id_f.astype(np.float16),
            "bmneg": np.broadcast_to(bm, (128, HPC)).copy(),
            "maskA16": maskA16_np, "id16": id16, "id64x2": id64x2_np,
            "ones_neg": ones_neg, "ones_col": ones_col_np,
            "ones_row": ones_row_np, "esel": esel_np,
        })
    return in_maps


def kernel(x, position_bias, W_attn, W_proj, _trace=False, _tmpdir=None):
    if "nc" not in _CACHE:
        _CACHE["nc"] = _build()
    nc = _CACHE["nc"]
    in_maps = _prep_inputs(x, position_bias, W_attn, W_proj)
    res = run_bass_kernel_spmd(nc, in_maps, list(range(N_CORES)),
                               trace=_trace, tmpdir=_tmpdir)
    if _trace:
        _CACHE["exec_time_ns"] = res.exec_time_ns
    out_full = np.empty((B, T, C), np.float32)
    for c in range(N_CORES):
        r = res.results[c]["out"].reshape(NSPAN, B, 64, C)
        for b in range(B):
            for Qs in range(NSPAN):
                out_full[b, Qs * 512 + 64 * c: Qs * 512 + 64 * (c + 1)] = r[Qs, b]
    return out_full
